# revision 2
# baseline (speedup 1.0000x reference)
"""Cross-attention (txt queries -> image kv) Trainium2 Bass kernel, v3.

v1 structure (data-parallel over batch, kv compaction, valid-first q
permutation, ymeanb blend, SBUF-resident X^T, softmax denominator fused as
V-tile column 64) with two fixes found by HW microbenchmarking:

  * QK^T runs at K=128 instead of K=64: Q^T is stored twice with the
    even-head / odd-head partition halves zeroed (QT2[...,0/1,:]), so each
    head's S^T matmul contracts over all 128 partitions (the extra half is
    zeros). K=64 matmuls HW-measure 234ns each (LDWEIGHTS not hidden);
    K=128 measure 117ns — the ~64 zero-row FLOPs are free.

  * psum->SBUF copies for K^T/V run on the Vector engine (DVE) rather than
    the Activation engine, keeping ACT free for the softmax exps (HW A/B:
    317us vs 329us per pass). An alternative schedule that interleaved the
    V projection into the first attention chain measured slower (380us) —
    the per-chunk PE->DVE->PE interlock it creates outweighs the fill; the
    v1 phase ordering is kept (SCHED="v1").

All matmuls bf16 (fp8 was measured end-to-end and rejected: with random
data the attention output is a cancelling average ~40x smaller than v, so
every fp8 path's ~3.5% quantization noise lands full-strength on the
output — each path alone costs 1.7-3.8e-2 of the 2e-2 error budget).
"""

import ml_dtypes
import numpy as np

import concourse.bass as bass
from concourse import bacc
import concourse.mybir as mybir
import concourse.tile as tile
from concourse.bass_utils import run_bass_kernel_spmd

B, I, J, D, E = 8, 512, 4096, 1024, 1024
H, DH = 16, 64
JC = J // 128   # 32
IC = I // 128   # 4
DC = D // 128   # 8
EC = E // 128   # 8
F32 = mybir.dt.float32
BF16 = mybir.dt.bfloat16
FP8 = mybir.dt.float8e4
DR = mybir.MatmulPerfMode.DoubleRow
BF = ml_dtypes.bfloat16
F8 = ml_dtypes.float8_e4m3

WS = 64.0          # host weight pre-scale (wq, wk, wv, wout)
OS = WS * WS       # output-projection result scale -> host divides by 4096

ETP_BUFS = 5
A_YIELD_EVERY = 4      # a_chunk emits a yield every N contraction matmuls
COPIES_ON_ACT = False  # psum->SBUF KT/V copies on Act (True) or DVE (False)
LOOKAHEAD = 2          # qk chunks issued ahead of pv in c_pair
SCHED = "v1"           # "v3": v_phase mixed into chain0; "v1": v1 ordering
JP_RESIDENT_MAX = 2816  # X^T fully SBUF-resident up to this padded kv length


def _mix(*streams):
    """Proportionally interleave step generators (PE program-order shaping)."""
    live = [[g, n, 0] for g, n in streams if n > 0]
    while live:
        g, n, done = min(live, key=lambda s: s[2] / s[1])
        try:
            next(g)
            for s in live:
                if s[0] is g:
                    s[2] += 1
        except StopIteration:
            live = [s for s in live if s[0] is not g]


def build_nc(jp, ip=I, repeat=1, reload_inputs=True):
    jcp = jp // 128
    nc = bacc.Bacc()

    txtT = nc.dram_tensor("txtT", [D, I], BF16, kind="ExternalInput")
    imT = nc.dram_tensor("imT", [D, jp], BF16, kind="ExternalInput")
    wq = nc.dram_tensor("wq", [D, E], BF16, kind="ExternalInput")
    wkv = nc.dram_tensor("wkv", [D, 2 * E], BF16, kind="ExternalInput")
    wout = nc.dram_tensor("wout", [E, D], BF16, kind="ExternalInput")
    kvmp = nc.dram_tensor("kvmp", [128, jcp], F32, kind="ExternalInput")
    qmp = nc.dram_tensor("qmp", [128, IC], F32, kind="ExternalInput")
    qmrow = nc.dram_tensor("qmrow", [1, I], BF16, kind="ExternalInput")
    omqrow = nc.dram_tensor("omqrow", [1, I], BF16, kind="ExternalInput")
    ymeanb = nc.dram_tensor("ymeanb", [1, D], BF16, kind="ExternalInput")
    boutr = nc.dram_tensor("boutr", [1, D], BF16, kind="ExternalInput")
    y = nc.dram_tensor("y", [I, D], BF16, kind="ExternalOutput")

    jblocks = []
    off = 0
    while off < jp:
        w = 512 if jp - off >= 512 else jp - off
        jblocks.append((off, w))
        off += w

    with tile.TileContext(nc) as tc:
        with (
            tc.tile_pool(name="wpool", bufs=1) as wpool,      # small resident
            tc.tile_pool(name="wslot", bufs=3) as wslot,      # wq/wv/wk live together
            tc.tile_pool(name="big", bufs=1) as big,          # imr, K^T, V, Q^T, O^T
            tc.tile_pool(name="work", bufs=3) as work,        # small working tiles
            tc.tile_pool(name="etp", bufs=ETP_BUFS) as etp,
            tc.tile_pool(name="ppool", bufs=3, space="PSUM") as ppool,   # 6 banks
            tc.tile_pool(name="accp", bufs=2, space="PSUM") as accp,     # 2 banks
        ):
            # ---- one-time allocations (shared across in-NEFF passes) ----
            ta = wpool.tile([128, DC, I], BF16)
            tar = txtT[:].rearrange("(dc p) i -> p dc i", p=128)
            wqr = wq[:].rearrange("(dc p) e -> p dc e", p=128)
            kvm_sb = wpool.tile([128, jcp], F32)
            qmp_sb = wpool.tile([128, IC], F32)
            qmrow_sb = wpool.tile([1, I], BF16)
            omqrow_sb = wpool.tile([1, I], BF16)
            ymeanb_sb = wpool.tile([1, D], BF16)
            boutr_sb = wpool.tile([1, D], BF16)
            ones64 = wpool.tile([1, DH], BF16)
            nc.vector.memset(ones64[:], 1.0)
            imr = big.tile([128, DC, jp], BF16)
            KT_sb = big.tile([128, EC, jp], BF16)
            n_vic = -(-ip // 128)         # i-blocks with any valid query
            ipp = n_vic * 128             # ip padded to whole i-blocks
            # QT2[:, ec, 0, :] has odd-head partitions (64:128) zeroed;
            # QT2[:, ec, 1, :] has even-head partitions (0:64) zeroed —
            # so per-head QK^T contracts over the full 128 partitions.
            QT2 = big.tile([128, EC, 2, ip], BF16)
            nc.vector.memset(QT2[:], 0.0)
            OT_sb = big.tile([128, EC, ipp], BF16)
            nc.vector.memset(OT_sb[:], 0.0)
            Yacc = big.tile([128, n_vic, 2, 512], BF16)
            V_sb = big.tile([128, jcp, H, DH + 1], BF16)
            imTr = imT[:].rearrange("(dc p) j -> p dc j", p=128)

            def emit_pass(first=True):
                # phase Q's operands go first — every DMA transfer
                # serializes through the DMA engines, so these gate PE
                wq_sb = wslot.tile([128, DC, E], BF16, tag="w")
                if first or reload_inputs:
                    nc.sync.dma_start(ta[:], tar[:])
                # wq split along E: phase-Q iteration ep only reads cols
                # [ep*256, (ep+1)*256) — first quarter unblocks PE early
                for q in range(4):
                    nc.scalar.dma_start(
                        wq_sb[:, :, q * 256 : (q + 1) * 256],
                        wqr[:, :, q * 256 : (q + 1) * 256],
                    )
                nc.sync.dma_start(kvm_sb[:], kvmp[:])
                nc.sync.dma_start(qmp_sb[:], qmp[:])
                nc.sync.dma_start(qmrow_sb[:], qmrow[:])
                nc.sync.dma_start(omqrow_sb[:], omqrow[:])
                nc.sync.dma_start(ymeanb_sb[:], ymeanb[:])
                nc.sync.dma_start(boutr_sb[:], boutr[:])
                nc.vector.tensor_copy(
                    V_sb[:, :, :, DH : DH + 1],
                    kvm_sb[:, :, None, None].to_broadcast([128, jcp, H, 1]),
                )
                wv_sb = wslot.tile([128, DC, E], BF16, tag="w")
                nc.scalar.dma_start(
                    wv_sb[:],
                    wkv[:, E : 2 * E].rearrange("(dc p) e -> p dc e", p=128),
                )
                if first or reload_inputs:
                    for off, w in [(o, min(512, jp - o)) for o in range(0, jp, 512)]:
                        nc.scalar.dma_start(
                            imr[:, :, off : off + w], imTr[:, :, off : off + w]
                        )
                wk_sb = wslot.tile([128, DC, E], BF16, tag="w")
                nc.sync.dma_start(
                    wk_sb[:], wkv[:, 0:E].rearrange("(dc p) e -> p dc e", p=128)
                )

                # ======= Phase Q: Q^T = Wq^T @ T^T, scaled 1/(8*WS) =========
                for ep in range(EC // 2):
                    ps = ppool.tile([128, 2, 512], F32, tag="sps", bufs=2)
                    for eh in range(2):
                        ec = 2 * ep + eh
                        for dc in range(DC):
                            nc.tensor.matmul(
                                ps[:, eh, 0:ip],
                                wq_sb[:, dc, ec * 128 : (ec + 1) * 128],
                                ta[:, dc, 0:ip],
                                start=(dc == 0),
                                stop=(dc == DC - 1),
                            )
                    # split per head parity into the two zero-padded copies
                    nc.vector.tensor_scalar_mul(
                        QT2[0:DH, 2 * ep : 2 * ep + 2, 0, 0:ip],
                        ps[0:DH, :, 0:ip], 0.125,
                    )
                    nc.vector.tensor_scalar_mul(
                        QT2[DH:128, 2 * ep : 2 * ep + 2, 1, 0:ip],
                        ps[DH:128, :, 0:ip], 0.125,
                    )

                # ===== Phase V: V' = (X Wv) * kvm (x WS), from resident X^T ==
                def v_phase():
                    for jc in range(jcp):
                        ps = ppool.tile([128, 2, 512], F32, tag="sps", bufs=2)
                        for eb in range(2):
                            for dc in range(DC):
                                nc.tensor.matmul(
                                    ps[:, eb, :],
                                    imr[:, dc, jc * 128 : (jc + 1) * 128],
                                    wv_sb[:, dc, eb * 512 : (eb + 1) * 512],
                                    start=(dc == 0),
                                    stop=(dc == DC - 1),
                                )
                        if COPIES_ON_ACT:
                            nc.scalar.activation(
                                V_sb[:, jc, :, 0:DH],
                                ps[:].rearrange("p b (h dh) -> p (b h) dh", dh=DH),
                                mybir.ActivationFunctionType.Copy,
                                scale=kvm_sb[:, jc : jc + 1],
                            )
                        else:
                            nc.vector.tensor_scalar_mul(
                                V_sb[:, jc, :, 0:DH],
                                ps[:].rearrange("p b (h dh) -> p (b h) dh", dh=DH),
                                kvm_sb[:, jc : jc + 1],
                            )
                        yield

                # ====== Main loop: K^T chunks pipelined against attention ====
                def a_chunk(ep):
                    # K^T rows for ec chunks 2ep, 2ep+1, from resident X^T
                    for off, w in jblocks:
                        ps = ppool.tile([128, 2, 512], F32, tag="aps", bufs=1)
                        for eh in range(2):
                            ec = 2 * ep + eh
                            for dc in range(DC):
                                nc.tensor.matmul(
                                    ps[:, eh, 0:w],
                                    wk_sb[:, dc, ec * 128 : (ec + 1) * 128],
                                    imr[:, dc, off : off + w],
                                    start=(dc == 0),
                                    stop=(dc == DC - 1),
                                )
                                if (dc + 1) % A_YIELD_EVERY == 0 and dc != DC - 1:
                                    yield
                            yield
                        if COPIES_ON_ACT:
                            nc.scalar.activation(
                                KT_sb[:, 2 * ep : 2 * ep + 2, off : off + w],
                                ps[:, :, 0:w],
                                mybir.ActivationFunctionType.Copy,
                            )
                        else:
                            nc.vector.tensor_copy(
                                KT_sb[:, 2 * ep : 2 * ep + 2, off : off + w],
                                ps[:, :, 0:w],
                            )

                def c_pair(hp, accs=None, lookahead=None):
                    if lookahead is None:
                        lookahead = LOOKAHEAD
                    vt = V_sb[:, :, 2 * hp : 2 * hp + 2, :]
                    if accs is None:
                        oacc_a = accp.tile([DH + 1, 512], F32, tag="oacc")
                        oacc_b = accp.tile([DH + 1, 512], F32, tag="oacc")
                    else:
                        oacc_a, oacc_b = accs

                    def qk(jc):
                        sps = ppool.tile([128, 2, 512], F32, tag="sps", bufs=2)
                        for hh in range(2):
                            nc.tensor.matmul(
                                sps[:, hh, 0:ip],
                                KT_sb[:, hp, jc * 128 : (jc + 1) * 128],
                                QT2[:, hp, hh, 0:ip],
                                start=True,
                                stop=True,
                            )
                        et = etp.tile([128, 2, ip], BF16, tag="et")
                        nc.scalar.activation(
                            et[:, :, 0:ip],
                            sps[:, :, 0:ip],
                            mybir.ActivationFunctionType.Exp,
                            scale=kvm_sb[:, jc : jc + 1],
                        )
                        return et

                    def pv(jc, et):
                        nc.tensor.matmul(
                            oacc_a[:, 0:ip],
                            vt[:, jc, 0, :],
                            et[:, 0, 0:ip],
                            start=(jc == 0),
                            stop=(jc == jcp - 1),
                        )
                        nc.tensor.matmul(
                            oacc_b[:, 0:ip],
                            vt[:, jc, 1, :],
                            et[:, 1, 0:ip],
                            start=(jc == 0),
                            stop=(jc == jcp - 1),
                        )

                    # software pipeline: QK^T issued `lookahead` chunks ahead of
                    # PV so PE never waits on the Activation engine's exp
                    ets = []
                    for jc in range(lookahead):
                        ets.append(qk(jc))
                        yield
                    for jc in range(lookahead, jcp):
                        ets.append(qk(jc))
                        pv(jc - lookahead, ets.pop(0))
                        yield
                    for k in range(lookahead):
                        pv(jcp - lookahead + k, ets.pop(0))
                        if k + 1 < lookahead:
                            yield

                    # denominator reciprocals first (DVE), then the PE broadcasts
                    recbs = []
                    for hh, oacc in ((0, oacc_a), (1, oacc_b)):
                        rec = work.tile([1, ip], F32, tag="rec")
                        nc.vector.reciprocal(rec[:, 0:ip], oacc[DH : DH + 1, 0:ip])
                        # fold the query-mask scale into the softmax recip so
                        # O^T comes out pre-scaled by qm
                        recb = work.tile([1, ip], BF16, tag="recb")
                        nc.vector.tensor_tensor(
                            recb[:, 0:ip], rec[:, 0:ip], qmrow_sb[:, 0:ip],
                            mybir.AluOpType.mult,
                        )
                        recbs.append(recb)
                    yield
                    for hh, oacc in ((0, oacc_a), (1, oacc_b)):
                        bps = ppool.tile([128, 2, 512], F32, tag="sps", bufs=2)
                        nc.tensor.matmul(
                            bps[0:DH, 0, 0:ip],
                            ones64[:],
                            recbs[hh][:, 0:ip],
                            start=True,
                            stop=True,
                        )
                        rb = work.tile([DH, ip], F32, tag="rb")
                        nc.vector.tensor_copy(rb[:, 0:ip], bps[0:DH, 0, 0:ip])
                        nc.vector.tensor_tensor(
                            OT_sb[hh * DH : (hh + 1) * DH, hp, 0:ip],
                            oacc[0:DH, 0:ip],
                            rb[:, 0:ip],
                            mybir.AluOpType.mult,
                        )
                        yield

                # prefetch the output-projection weights into wq's slot
                wo_sb = wslot.tile([128, EC, D], BF16, tag="w")
                nc.gpsimd.dma_start(
                    wo_sb[:], wout[:].rearrange("(ec p) d -> p ec d", p=128)
                )

                def c_chain(hp0):
                    yield from c_pair(hp0)
                    yield from c_pair(hp0 + 1)

                n_a = 2 * (DC // A_YIELD_EVERY) * len(jblocks)
                n_c = 2 * (jcp + 4)           # yields per c_chain
                if SCHED == "v3":
                    # a_chunk(0) first so chain 0 can start; then chain 0
                    # overlaps the V projection AND a_chunk(1) (3-way mix).
                    # v_phase is primed 3 chunks ahead and weighted 1:1 with
                    # the chain so V'(jc) is always emitted before chain0's
                    # PV(jc) (emission-order inversion there would deadlock
                    # the PE queue).
                    for _ in a_chunk(0):
                        pass
                    vg = v_phase()
                    for _ in range(3):
                        next(vg)
                    _mix((vg, n_c), (a_chunk(1), n_a), (c_chain(0), n_c))
                    for ep in range(2, EC // 2):
                        _mix((a_chunk(ep), n_a), (c_chain(2 * ep - 2), n_c))
                else:
                    _mix((v_phase(), jcp), (a_chunk(0), n_a))
                    for ep in range(1, EC // 2):
                        _mix((a_chunk(ep), n_a), (c_chain(2 * ep - 2), n_c))

                # drain the last two attention pairs, interleaved with the
                # output projection for the heads that are already finished
                def d_partial():
                    for ic in range(n_vic):
                        dps = ppool.tile([128, 2, 512], F32, tag="aps", bufs=1)
                        for db in range(2):
                            for ec in range(EC - 2):
                                nc.tensor.matmul(
                                    dps[:, db, :],
                                    OT_sb[:, ec, ic * 128 : (ic + 1) * 128],
                                    wo_sb[:, ec, db * 512 : (db + 1) * 512],
                                    start=(ec == 0),
                                    stop=False,
                                )
                            nc.tensor.matmul(
                                dps[:, db, :],
                                omqrow_sb[:, ic * 128 : (ic + 1) * 128],
                                ymeanb_sb[:, db * 512 : (db + 1) * 512],
                                start=False,
                                stop=False,
                            )
                            nc.tensor.matmul(
                                dps[:, db, :],
                                qmrow_sb[:, ic * 128 : (ic + 1) * 128],
                                boutr_sb[:, db * 512 : (db + 1) * 512],
                                start=False,
                                stop=True,
                            )
                            yield
                        nc.vector.tensor_copy(Yacc[:, ic, :, :], dps[:])
                        yield

                _mix((c_chain(EC - 2), n_c), (d_partial(), 3 * n_vic))

                # ============ Phase D: finish Y (ec 6,7) + blend ==============
                ic_order = sorted(range(IC), key=lambda ic: ic * 128 < ip)
                for ic in ic_order:
                    has_valid = ic * 128 < ip
                    if has_valid:
                        yps = ppool.tile([128, 2, 512], F32, tag="sps", bufs=2)
                        for db in range(2):
                            for ec in range(EC - 2, EC):
                                nc.tensor.matmul(
                                    yps[:, db, :],
                                    OT_sb[:, ec, ic * 128 : (ic + 1) * 128],
                                    wo_sb[:, ec, db * 512 : (db + 1) * 512],
                                    start=(ec == EC - 2),
                                    stop=(ec == EC - 1),
                                )
                        for db in range(2):
                            y1 = work.tile([128, 512], BF16, tag="y1", bufs=3)
                            nc.vector.tensor_tensor(
                                y1[:], yps[:, db, :], Yacc[:, ic, db, :],
                                mybir.AluOpType.add,
                            )
                            nc.sync.dma_start(
                                y[ic * 128 : (ic + 1) * 128,
                                  db * 512 : (db + 1) * 512],
                                y1[:],
                            )
                    else:
                        bb_a = accp.tile([128, 512], F32, tag="oacc")
                        bb_b = accp.tile([128, 512], F32, tag="oacc")
                        for db, bbps in ((0, bb_a), (1, bb_b)):
                            nc.tensor.matmul(
                                bbps[:],
                                omqrow_sb[:, ic * 128 : (ic + 1) * 128],
                                ymeanb_sb[:, db * 512 : (db + 1) * 512],
                                start=True,
                                stop=False,
                            )
                            nc.tensor.matmul(
                                bbps[:],
                                qmrow_sb[:, ic * 128 : (ic + 1) * 128],
                                boutr_sb[:, db * 512 : (db + 1) * 512],
                                start=False,
                                stop=True,
                            )
                        for db, bb in ((0, bb_a), (1, bb_b)):
                            y1 = work.tile([128, 512], BF16, tag="y1", bufs=3)
                            nc.vector.tensor_copy(y1[:], bb[:])
                            nc.sync.dma_start(
                                y[ic * 128 : (ic + 1) * 128,
                                  db * 512 : (db + 1) * 512],
                                y1[:],
                            )

            for it in range(repeat):
                emit_pass(first=(it == 0))

    nc.compile()
    return nc


def build_nc_stream(jp=J, ip=I):
    """Streaming fallback for jp > JP_RESIDENT_MAX (X^T doesn't fit SBUF):
    the original 5-pass streaming kernel, V through a DRAM round-trip."""
    jcp = jp // 128
    nc = bacc.Bacc()

    txtT = nc.dram_tensor("txtT", [D, I], BF16, kind="ExternalInput")
    imT = nc.dram_tensor("imT", [D, jp], BF16, kind="ExternalInput")
    wq = nc.dram_tensor("wq", [D, E], BF16, kind="ExternalInput")
    wkv = nc.dram_tensor("wkv", [D, 2 * E], BF16, kind="ExternalInput")
    wout = nc.dram_tensor("wout", [E, D], BF16, kind="ExternalInput")
    kvmp = nc.dram_tensor("kvmp", [128, jcp], F32, kind="ExternalInput")
    qmp = nc.dram_tensor("qmp", [128, IC], F32, kind="ExternalInput")
    qmrow = nc.dram_tensor("qmrow", [1, I], BF16, kind="ExternalInput")
    omqrow = nc.dram_tensor("omqrow", [1, I], BF16, kind="ExternalInput")
    ymeanb = nc.dram_tensor("ymeanb", [1, D], BF16, kind="ExternalInput")
    boutr = nc.dram_tensor("boutr", [1, D], BF16, kind="ExternalInput")
    y = nc.dram_tensor("y", [I, D], F32, kind="ExternalOutput")
    vdr = nc.dram_tensor("vdr", [jcp, 128, H, DH], BF16, kind="Internal")

    imTr = imT[:].rearrange("(dc p) j -> p dc j", p=128)

    with tile.TileContext(nc) as tc:
        with (
            tc.tile_pool(name="wpool", bufs=1) as wpool,
            tc.tile_pool(name="wslot", bufs=1) as wslot,
            tc.tile_pool(name="big", bufs=1) as big,
            tc.tile_pool(name="stream", bufs=2) as stream,
            tc.tile_pool(name="work", bufs=3) as work,
            tc.tile_pool(name="etp", bufs=7) as etp,
            tc.tile_pool(name="ppool", bufs=2, space="PSUM") as ppool,
            tc.tile_pool(name="accp", bufs=3, space="PSUM") as accp,
            tc.tile_pool(name="bcp", bufs=1, space="PSUM") as bcp,
        ):
            kvm_sb = wpool.tile([128, jcp], F32)
            nc.sync.dma_start(kvm_sb[:], kvmp[:])
            qmp_sb = wpool.tile([128, IC], F32)
            nc.sync.dma_start(qmp_sb[:], qmp[:])
            qmrow_sb = wpool.tile([1, I], BF16)
            nc.sync.dma_start(qmrow_sb[:], qmrow[:])
            omqrow_sb = wpool.tile([1, I], BF16)
            nc.sync.dma_start(omqrow_sb[:], omqrow[:])
            ymeanb_sb = wpool.tile([1, D], BF16)
            nc.sync.dma_start(ymeanb_sb[:], ymeanb[:])
            boutr_sb = wpool.tile([1, D], BF16)
            nc.sync.dma_start(boutr_sb[:], boutr[:])
            ones64 = wpool.tile([1, DH], BF16)
            nc.vector.memset(ones64[:], 1.0)

            KT_sb = big.tile([128, EC, jp], BF16)
            QT_sb = big.tile([128, EC, I], BF16)
            OT_sb = big.tile([128, EC, I], BF16)
            nc.vector.memset(OT_sb[:], 0.0)

            wq_sb = wslot.tile([128, DC, E], BF16, tag="w")
            nc.gpsimd.dma_start(wq_sb[:], wq[:].rearrange("(dc p) e -> p dc e", p=128))
            ta = stream.tile([128, DC, I], BF16, tag="im")
            nc.gpsimd.dma_start(ta[:], txtT[:].rearrange("(dc p) i -> p dc i", p=128))
            for ep in range(EC // 2):
                ps = ppool.tile([128, 2, 512], F32, tag="s2")
                for eh in range(2):
                    ec = 2 * ep + eh
                    for dc in range(DC):
                        nc.tensor.matmul(
                            ps[:, eh, 0:ip],
                            wq_sb[:, dc, ec * 128 : (ec + 1) * 128],
                            ta[:, dc, 0:ip],
                            start=(dc == 0),
                            stop=(dc == DC - 1),
                        )
                nc.vector.tensor_scalar_mul(
                    QT_sb[:, 2 * ep : 2 * ep + 2, 0:ip], ps[:, :, 0:ip], 0.125
                )

            wv_sb = wslot.tile([128, DC, E], BF16, tag="w")
            nc.gpsimd.dma_start(
                wv_sb[:], wkv[:, E : 2 * E].rearrange("(dc p) e -> p dc e", p=128)
            )
            wk_sb = wslot.tile([128, DC, E], BF16, tag="w")
            nc.gpsimd.dma_start(
                wk_sb[:], wkv[:, 0:E].rearrange("(dc p) e -> p dc e", p=128)
            )

            jblocks = []
            off = 0
            while off < jp:
                w = 512 if jp - off >= 512 else jp - off
                jblocks.append((off, w))
                off += w

            def b_all():
                for off, w in jblocks:
                    imb = stream.tile([128, DC, 512], BF16, tag="ima")
                    nc.gpsimd.dma_start(
                        imb[:, :, 0:w], imTr[:, :, off : off + w]
                    )
                    for jh in range(w // 128):
                        jc = off // 128 + jh
                        ps = ppool.tile([128, 2, 512], F32, tag="s2")
                        for eb in range(2):
                            for dc in range(DC):
                                nc.tensor.matmul(
                                    ps[:, eb, :],
                                    imb[:, dc, jh * 128 : (jh + 1) * 128],
                                    wv_sb[:, dc, eb * 512 : (eb + 1) * 512],
                                    start=(dc == 0),
                                    stop=(dc == DC - 1),
                                )
                        vtmp = work.tile([128, H, DH], BF16, tag="vtmp")
                        nc.vector.tensor_scalar_mul(
                            vtmp[:],
                            ps[:].rearrange("p b (h dh) -> p (b h) dh", dh=DH),
                            kvm_sb[:, jc : jc + 1],
                        )
                        nc.sync.dma_start(vdr[jc, :, :, :], vtmp[:])

            def a_chunk(ep):
                for off, w in jblocks:
                    ima = stream.tile([128, DC, 512], BF16, tag="ima")
                    nc.gpsimd.dma_start(ima[:, :, 0:w], imTr[:, :, off : off + w])
                    ps = ppool.tile([128, 2, 512], F32, tag="s2")
                    for eh in range(2):
                        ec = 2 * ep + eh
                        for dc in range(DC):
                            nc.tensor.matmul(
                                ps[:, eh, 0:w],
                                wk_sb[:, dc, ec * 128 : (ec + 1) * 128],
                                ima[:, dc, 0:w],
                                start=(dc == 0),
                                stop=(dc == DC - 1),
                            )
                    nc.vector.tensor_copy(
                        KT_sb[:, 2 * ep : 2 * ep + 2, off : off + w],
                        ps[:, :, 0:w],
                    )

            def c_pair(hp):
                vtt = stream.tile([128, jcp, 2, DH + 1], BF16, tag="vt")
                nc.vector.tensor_copy(
                    vtt[:, :, 0, DH : DH + 1], kvm_sb[:, :, None]
                )
                nc.vector.tensor_copy(
                    vtt[:, :, 1, DH : DH + 1], kvm_sb[:, :, None]
                )
                for hh in range(2):
                    nc.sync.dma_start(
                        vtt[:, :, hh, 0:DH],
                        vdr[:, :, 2 * hp + hh, :].rearrange("jc p dh -> p jc dh"),
                    )
                vt = vtt
                oacc_a = accp.tile([DH + 1, 512], F32, tag="oacc")
                oacc_b = accp.tile([DH + 1, 512], F32, tag="oacc")
                for jc in range(jcp):
                    sps = ppool.tile([128, 2, 512], F32, tag="s2")
                    nc.tensor.matmul(
                        sps[:, 0, 0:ip],
                        KT_sb[0:DH, hp, jc * 128 : (jc + 1) * 128],
                        QT_sb[0:DH, hp, 0:ip],
                        start=True,
                        stop=True,
                    )
                    nc.tensor.matmul(
                        sps[:, 1, 0:ip],
                        KT_sb[DH:128, hp, jc * 128 : (jc + 1) * 128],
                        QT_sb[DH:128, hp, 0:ip],
                        start=True,
                        stop=True,
                    )
                    et = etp.tile([128, 2, ip], BF16, tag="et")
                    nc.scalar.activation(
                        et[:, :, 0:ip],
                        sps[:, :, 0:ip],
                        mybir.ActivationFunctionType.Exp,
                        scale=kvm_sb[:, jc : jc + 1],
                    )
                    nc.tensor.matmul(
                        oacc_a[:, 0:ip],
                        vt[:, jc, 0, :],
                        et[:, 0, 0:ip],
                        start=(jc == 0),
                        stop=(jc == jcp - 1),
                    )
                    nc.tensor.matmul(
                        oacc_b[:, 0:ip],
                        vt[:, jc, 1, :],
                        et[:, 1, 0:ip],
                        start=(jc == 0),
                        stop=(jc == jcp - 1),
                    )
                for hh, oacc in ((0, oacc_a), (1, oacc_b)):
                    rec = work.tile([1, ip], F32, tag="rec")
                    nc.vector.reciprocal(rec[:, 0:ip], oacc[DH : DH + 1, 0:ip])
                    recb = work.tile([1, ip], BF16, tag="recb")
                    nc.vector.tensor_copy(recb[:, 0:ip], rec[:, 0:ip])
                    bps = bcp.tile([DH, 512], F32, tag="bc")
                    nc.tensor.matmul(
                        bps[:, 0:ip],
                        ones64[:],
                        recb[:, 0:ip],
                        start=True,
                        stop=True,
                    )
                    rb = work.tile([DH, ip], F32, tag="rb")
                    nc.vector.tensor_copy(rb[:, 0:ip], bps[:, 0:ip])
                    nc.vector.tensor_tensor(
                        OT_sb[hh * DH : (hh + 1) * DH, hp, 0:ip],
                        oacc[0:DH, 0:ip],
                        rb[:, 0:ip],
                        mybir.AluOpType.mult,
                    )

            b_all()
            for ep in range(EC // 2):
                a_chunk(ep)
                c_pair(2 * ep)
                c_pair(2 * ep + 1)

            wo_sb = wslot.tile([128, DC, E], BF16, tag="w")
            nc.gpsimd.dma_start(
                wo_sb[:], wout[:].rearrange("(ec p) d -> p ec d", p=128)
            )
            for ic in range(IC):
                has_valid = ic * 128 < ip
                if has_valid:
                    yps = ppool.tile([128, 2, 512], F32, tag="s2")
                    for db in range(2):
                        for ec in range(EC):
                            nc.tensor.matmul(
                                yps[:, db, :],
                                OT_sb[:, ec, ic * 128 : (ic + 1) * 128],
                                wo_sb[:, ec, db * 512 : (db + 1) * 512],
                                start=(ec == 0),
                                stop=(ec == EC - 1),
                            )
                bb_a = accp.tile([128, 512], F32, tag="oacc")
                bb_b = accp.tile([128, 512], F32, tag="oacc")
                for db, bbps in ((0, bb_a), (1, bb_b)):
                    nc.tensor.matmul(
                        bbps[:],
                        omqrow_sb[:, ic * 128 : (ic + 1) * 128],
                        ymeanb_sb[:, db * 512 : (db + 1) * 512],
                        start=True,
                        stop=False,
                    )
                    nc.tensor.matmul(
                        bbps[:],
                        qmrow_sb[:, ic * 128 : (ic + 1) * 128],
                        boutr_sb[:, db * 512 : (db + 1) * 512],
                        start=False,
                        stop=True,
                    )
                y1 = work.tile([128, 2, 512], F32, tag="y1")
                if has_valid:
                    nc.vector.tensor_scalar_mul(
                        y1[:], yps[:], qmp_sb[:, ic : ic + 1]
                    )
                    nc.vector.tensor_tensor(
                        y1[:, 0, :], bb_a[:], y1[:, 0, :], mybir.AluOpType.add
                    )
                    nc.vector.tensor_tensor(
                        y1[:, 1, :], bb_b[:], y1[:, 1, :], mybir.AluOpType.add
                    )
                else:
                    nc.vector.tensor_copy(y1[:, 0, :], bb_a[:])
                    nc.vector.tensor_copy(y1[:, 1, :], bb_b[:])
                nc.sync.dma_start(
                    y[ic * 128 : (ic + 1) * 128, :],
                    y1[:].rearrange("p b d -> p (b d)"),
                )

    nc.compile()
    return nc


_NC_CACHE = {}


def _get_nc(jp, ip=I, repeat=1):
    key = (jp, ip, repeat)
    if key not in _NC_CACHE:
        if jp <= JP_RESIDENT_MAX:
            _NC_CACHE[key] = build_nc(jp, ip, repeat)
        else:
            assert repeat == 1
            _NC_CACHE[key] = build_nc_stream(jp, ip)
    return _NC_CACHE[key]


def _q8(x):
    return np.clip(np.asarray(x, np.float32), -240.0, 240.0).astype(F8)


def prep_inputs(txt, image, kv_mask, q_mask, Wq, Wkv, Wout, bout):
    f32 = np.float32
    Wq = np.asarray(Wq, dtype=f32)
    Wkv = np.asarray(Wkv, dtype=f32)
    Wout = np.asarray(Wout, dtype=f32)
    bout = np.asarray(bout, dtype=f32)
    kvc = kv_mask.sum(axis=1).max()
    qc = q_mask.sum(axis=1).max()
    jp = max(512, int(-(-kvc // 128)) * 128)
    ip = max(256, int(-(-qc // 16)) * 16)
    jcp = jp // 128
    fast = jp <= JP_RESIDENT_MAX
    wq_s = Wq.astype(BF)
    wkv_s = Wkv.astype(BF)
    wout_s = Wout.astype(BF)
    in_maps = []
    perms = []
    for b in range(B):
        kvm = kv_mask[b].astype(bool)
        qm = q_mask[b].astype(bool)
        nkv = int(kvm.sum())
        imTc = np.zeros((D, jp), dtype=BF)
        imTc[:, :nkv] = np.ascontiguousarray(image[b][kvm].T).astype(BF)
        kvmp = np.zeros(jp, dtype=f32)
        kvmp[:nkv] = 1.0
        perm = np.argsort(~qm, kind="stable")
        perms.append(perm)
        qmperm = qm[perm].astype(f32)
        xmean = image[b].astype(f32).mean(axis=0)
        vmean = xmean @ Wkv[:, E:]
        ymb = vmean @ Wout + bout
        blend_s = 1.0
        txtTc = np.ascontiguousarray(txt[b][perm].T)
        in_maps.append(
            {
                "txtT": txtTc.astype(BF),
                "imT": imTc,
                "wq": wq_s,
                "wkv": wkv_s,
                "wout": wout_s,
                "kvmp": np.ascontiguousarray(kvmp.reshape(jcp, 128).T),
                "qmp": np.ascontiguousarray(qmperm.reshape(IC, 128).T),
                "qmrow": qmperm[None, :].astype(BF),
                "omqrow": (1.0 - qmperm)[None, :].astype(BF),
                "ymeanb": (ymb * blend_s)[None, :].astype(BF),
                "boutr": (bout * blend_s)[None, :].astype(BF),
            }
        )
    return in_maps, perms, jp, ip


def run(inputs, trace=False):
    in_maps, perms, jp, ip = prep_inputs(**inputs)
    nc = _get_nc(jp, ip)
    res = run_bass_kernel_spmd(
        nc, in_maps, core_ids=list(range(B)), trace=trace,
        **({"trace_cores": [0]} if trace else {}),
    )
    out = np.empty((B, I, D), dtype=np.float32)
    descale = 1.0
    for b in range(B):
        out[b][perms[b]] = np.asarray(res.results[b]["y"], np.float32) * descale
    return out, res


def kernel(**inputs):
    out, _ = run(inputs, trace=False)
    return out


# revision 3
# speedup vs baseline: 1.0056x; 1.0056x over previous
"""Cross-attention (txt queries -> image kv) Trainium2 Bass kernel, v3.

v1 structure (data-parallel over batch, kv compaction, valid-first q
permutation, ymeanb blend, SBUF-resident X^T, softmax denominator fused as
V-tile column 64) with two fixes found by HW microbenchmarking:

  * QK^T runs at K=128 instead of K=64: Q^T is stored twice with the
    even-head / odd-head partition halves zeroed (QT2[...,0/1,:]), so each
    head's S^T matmul contracts over all 128 partitions (the extra half is
    zeros). K=64 matmuls HW-measure 234ns each (LDWEIGHTS not hidden);
    K=128 measure 117ns — the ~64 zero-row FLOPs are free.

  * psum->SBUF copies for K^T/V run on the Vector engine (DVE) rather than
    the Activation engine, keeping ACT free for the softmax exps (HW A/B:
    317us vs 329us per pass). An alternative schedule that interleaved the
    V projection into the first attention chain measured slower (380us) —
    the per-chunk PE->DVE->PE interlock it creates outweighs the fill; the
    v1 phase ordering is kept (SCHED="v1").

All matmuls bf16 (fp8 was measured end-to-end and rejected: with random
data the attention output is a cancelling average ~40x smaller than v, so
every fp8 path's ~3.5% quantization noise lands full-strength on the
output — each path alone costs 1.7-3.8e-2 of the 2e-2 error budget).
"""

import ml_dtypes
import numpy as np

import concourse.bass as bass
from concourse import bacc
import concourse.mybir as mybir
import concourse.tile as tile
from concourse.bass_utils import run_bass_kernel_spmd

B, I, J, D, E = 8, 512, 4096, 1024, 1024
H, DH = 16, 64
JC = J // 128   # 32
IC = I // 128   # 4
DC = D // 128   # 8
EC = E // 128   # 8
F32 = mybir.dt.float32
BF16 = mybir.dt.bfloat16
FP8 = mybir.dt.float8e4
DR = mybir.MatmulPerfMode.DoubleRow
BF = ml_dtypes.bfloat16
F8 = ml_dtypes.float8_e4m3

WS = 64.0          # host weight pre-scale (wq, wk, wv, wout)
OS = WS * WS       # output-projection result scale -> host divides by 4096

SCHED = "v1"           # "x2": 2-chunk fused exps; "dual"; "v1"; ...
ETP_BUFS = 8 if SCHED == "dual" else (3 if SCHED == "x2" else 5)
A_YIELD_EVERY = 4      # a_chunk emits a yield every N contraction matmuls
COPIES_ON_ACT = False  # psum->SBUF KT/V copies on Act (True) or DVE (False)
LOOKAHEAD = 2          # qk chunks issued ahead of pv in c_pair
ACCP_BUFS = 4 if SCHED == "dual" else 2
SPS4 = SCHED == "x2"   # qk psum is one [128,4,512] tile (4 banks, bufs=1)
PPOOL_BUFS = 2 if SCHED == "dual" else 3
JP_RESIDENT_MAX = 2816  # X^T fully SBUF-resident up to this padded kv length


def _mix(*streams):
    """Proportionally interleave step generators (PE program-order shaping)."""
    live = [[g, n, 0] for g, n in streams if n > 0]
    while live:
        g, n, done = min(live, key=lambda s: s[2] / s[1])
        try:
            next(g)
            for s in live:
                if s[0] is g:
                    s[2] += 1
        except StopIteration:
            live = [s for s in live if s[0] is not g]


APS_TAG = "sps" if SCHED == "dual" else "aps"
APS_BUFS = 2 if SCHED == "dual" else 1
# In x2 mode the remaining non-qk psum users share the aps slot (bufs=1).


def build_nc(jp, ip=I, repeat=1, reload_inputs=True):
    jcp = jp // 128
    nc = bacc.Bacc()

    txtT = nc.dram_tensor("txtT", [D, I], BF16, kind="ExternalInput")
    imT = nc.dram_tensor("imT", [D, jp], BF16, kind="ExternalInput")
    wq = nc.dram_tensor("wq", [D, E], BF16, kind="ExternalInput")
    wkv = nc.dram_tensor("wkv", [D, 2 * E], BF16, kind="ExternalInput")
    wout = nc.dram_tensor("wout", [E, D], BF16, kind="ExternalInput")
    kvmp = nc.dram_tensor("kvmp", [128, jcp], F32, kind="ExternalInput")
    qmp = nc.dram_tensor("qmp", [128, IC], F32, kind="ExternalInput")
    qmrow = nc.dram_tensor("qmrow", [1, I], BF16, kind="ExternalInput")
    omqrow = nc.dram_tensor("omqrow", [1, I], BF16, kind="ExternalInput")
    ymeanb = nc.dram_tensor("ymeanb", [1, D], BF16, kind="ExternalInput")
    boutr = nc.dram_tensor("boutr", [1, D], BF16, kind="ExternalInput")
    y = nc.dram_tensor("y", [I, D], BF16, kind="ExternalOutput")

    jblocks = []
    off = 0
    while off < jp:
        w = 512 if jp - off >= 512 else jp - off
        jblocks.append((off, w))
        off += w

    with tile.TileContext(nc) as tc:
        with (
            tc.tile_pool(name="wpool", bufs=1) as wpool,      # small resident
            tc.tile_pool(name="wslot", bufs=3) as wslot,      # wq/wv/wk live together
            tc.tile_pool(name="big", bufs=1) as big,          # imr, K^T, V, Q^T, O^T
            tc.tile_pool(name="work", bufs=3) as work,        # small working tiles
            tc.tile_pool(name="etp", bufs=ETP_BUFS) as etp,
            tc.tile_pool(name="ppool", bufs=PPOOL_BUFS, space="PSUM") as ppool,
            tc.tile_pool(name="accp", bufs=ACCP_BUFS, space="PSUM") as accp,
        ):
            # ---- one-time allocations (shared across in-NEFF passes) ----
            ta = wpool.tile([128, DC, I], BF16)
            tar = txtT[:].rearrange("(dc p) i -> p dc i", p=128)
            wqr = wq[:].rearrange("(dc p) e -> p dc e", p=128)
            kvm_sb = wpool.tile([128, jcp], F32)
            qmp_sb = wpool.tile([128, IC], F32)
            qmrow_sb = wpool.tile([1, I], BF16)
            omqrow_sb = wpool.tile([1, I], BF16)
            ymeanb_sb = wpool.tile([1, D], BF16)
            boutr_sb = wpool.tile([1, D], BF16)
            ones64 = wpool.tile([1, DH], BF16)
            nc.vector.memset(ones64[:], 1.0)
            imr = big.tile([128, DC, jp], BF16)
            KT_sb = big.tile([128, EC, jp], BF16)
            n_vic = -(-ip // 128)         # i-blocks with any valid query
            ipp = n_vic * 128             # ip padded to whole i-blocks
            # QT2[:, ec, 0, :] has odd-head partitions (64:128) zeroed;
            # QT2[:, ec, 1, :] has even-head partitions (0:64) zeroed —
            # so per-head QK^T contracts over the full 128 partitions.
            QT2 = big.tile([128, EC, 2, ip], BF16)
            nc.vector.memset(QT2[:], 0.0)
            OT_sb = big.tile([128, EC, ipp], BF16)
            nc.vector.memset(OT_sb[:], 0.0)
            Yacc = None if SCHED == "dual" else big.tile([128, n_vic, 2, 512], BF16)
            V_sb = big.tile([128, jcp, H, DH + 1], BF16)
            imTr = imT[:].rearrange("(dc p) j -> p dc j", p=128)

            def qpool(shape):
                if SPS4:
                    qp = ppool.tile(shape, F32, tag="aps", bufs=1, name="qp")
                else:
                    qp = ppool.tile(shape, F32, tag="sps", bufs=2, name="qp")
                return qp

            def emit_pass(first=True):
                # phase Q's operands go first — every DMA transfer
                # serializes through the DMA engines, so these gate PE
                wq_sb = wslot.tile([128, DC, E], BF16, tag="w")
                if first or reload_inputs:
                    nc.sync.dma_start(ta[:], tar[:])
                # wq split along E: phase-Q iteration ep only reads cols
                # [ep*256, (ep+1)*256) — first quarter unblocks PE early
                for q in range(4):
                    nc.scalar.dma_start(
                        wq_sb[:, :, q * 256 : (q + 1) * 256],
                        wqr[:, :, q * 256 : (q + 1) * 256],
                    )
                nc.sync.dma_start(kvm_sb[:], kvmp[:])
                nc.sync.dma_start(qmp_sb[:], qmp[:])
                nc.sync.dma_start(qmrow_sb[:], qmrow[:])
                nc.sync.dma_start(omqrow_sb[:], omqrow[:])
                nc.sync.dma_start(ymeanb_sb[:], ymeanb[:])
                nc.sync.dma_start(boutr_sb[:], boutr[:])
                nc.vector.tensor_copy(
                    V_sb[:, :, :, DH : DH + 1],
                    kvm_sb[:, :, None, None].to_broadcast([128, jcp, H, 1]),
                )
                wv_sb = wslot.tile([128, DC, E], BF16, tag="w")
                nc.scalar.dma_start(
                    wv_sb[:],
                    wkv[:, E : 2 * E].rearrange("(dc p) e -> p dc e", p=128),
                )
                if first or reload_inputs:
                    for off, w in [(o, min(512, jp - o)) for o in range(0, jp, 512)]:
                        nc.scalar.dma_start(
                            imr[:, :, off : off + w], imTr[:, :, off : off + w]
                        )
                wk_sb = wslot.tile([128, DC, E], BF16, tag="w")
                nc.sync.dma_start(
                    wk_sb[:], wkv[:, 0:E].rearrange("(dc p) e -> p dc e", p=128)
                )

                # ======= Phase Q: Q^T = Wq^T @ T^T, scaled 1/(8*WS) =========
                for ep in range(EC // 2):
                    ps = qpool([128, 2, 512])
                    for eh in range(2):
                        ec = 2 * ep + eh
                        for dc in range(DC):
                            nc.tensor.matmul(
                                ps[:, eh, 0:ip],
                                wq_sb[:, dc, ec * 128 : (ec + 1) * 128],
                                ta[:, dc, 0:ip],
                                start=(dc == 0),
                                stop=(dc == DC - 1),
                            )
                    # split per head parity into the two zero-padded copies
                    nc.vector.tensor_scalar_mul(
                        QT2[0:DH, 2 * ep : 2 * ep + 2, 0, 0:ip],
                        ps[0:DH, :, 0:ip], 0.125,
                    )
                    nc.vector.tensor_scalar_mul(
                        QT2[DH:128, 2 * ep : 2 * ep + 2, 1, 0:ip],
                        ps[DH:128, :, 0:ip], 0.125,
                    )

                # ===== Phase V: V' = (X Wv) * kvm (x WS), from resident X^T ==
                def v_phase():
                    for jc in range(jcp):
                        ps = qpool([128, 2, 512])
                        for eb in range(2):
                            for dc in range(DC):
                                nc.tensor.matmul(
                                    ps[:, eb, :],
                                    imr[:, dc, jc * 128 : (jc + 1) * 128],
                                    wv_sb[:, dc, eb * 512 : (eb + 1) * 512],
                                    start=(dc == 0),
                                    stop=(dc == DC - 1),
                                )
                        if COPIES_ON_ACT:
                            nc.scalar.activation(
                                V_sb[:, jc, :, 0:DH],
                                ps[:].rearrange("p b (h dh) -> p (b h) dh", dh=DH),
                                mybir.ActivationFunctionType.Copy,
                                scale=kvm_sb[:, jc : jc + 1],
                            )
                        else:
                            nc.vector.tensor_scalar_mul(
                                V_sb[:, jc, :, 0:DH],
                                ps[:].rearrange("p b (h dh) -> p (b h) dh", dh=DH),
                                kvm_sb[:, jc : jc + 1],
                            )
                        yield

                # ====== Main loop: K^T chunks pipelined against attention ====
                def a_chunk(ep):
                    # K^T rows for ec chunks 2ep, 2ep+1, from resident X^T
                    for off, w in jblocks:
                        ps = ppool.tile([128, 2, 512], F32, tag=APS_TAG, bufs=APS_BUFS)
                        for eh in range(2):
                            ec = 2 * ep + eh
                            for dc in range(DC):
                                nc.tensor.matmul(
                                    ps[:, eh, 0:w],
                                    wk_sb[:, dc, ec * 128 : (ec + 1) * 128],
                                    imr[:, dc, off : off + w],
                                    start=(dc == 0),
                                    stop=(dc == DC - 1),
                                )
                                if (dc + 1) % A_YIELD_EVERY == 0 and dc != DC - 1:
                                    yield
                            yield
                        if COPIES_ON_ACT:
                            nc.scalar.activation(
                                KT_sb[:, 2 * ep : 2 * ep + 2, off : off + w],
                                ps[:, :, 0:w],
                                mybir.ActivationFunctionType.Copy,
                            )
                        else:
                            nc.vector.tensor_copy(
                                KT_sb[:, 2 * ep : 2 * ep + 2, off : off + w],
                                ps[:, :, 0:w],
                            )

                def c_pair(hp, accs=None, lookahead=None):
                    if lookahead is None:
                        lookahead = LOOKAHEAD
                    vt = V_sb[:, :, 2 * hp : 2 * hp + 2, :]
                    if accs is None:
                        oacc_a = accp.tile([DH + 1, 512], F32, tag="oacc")
                        oacc_b = accp.tile([DH + 1, 512], F32, tag="oacc")
                    else:
                        oacc_a, oacc_b = accs

                    def qk(jc):
                        sps = ppool.tile([128, 2, 512], F32, tag="sps", bufs=2)
                        for hh in range(2):
                            nc.tensor.matmul(
                                sps[:, hh, 0:ip],
                                KT_sb[:, hp, jc * 128 : (jc + 1) * 128],
                                QT2[:, hp, hh, 0:ip],
                                start=True,
                                stop=True,
                            )
                        et = etp.tile([128, 2, ip], BF16, tag="et")
                        # no kvm scale: compacted K^T is zero at padded j, so
                        # S=0 there and exp(0)=1 is annihilated by V'=0 plus
                        # the kvm denominator column (HW-verified bit-equal)
                        nc.scalar.activation(
                            et[:, :, 0:ip],
                            sps[:, :, 0:ip],
                            mybir.ActivationFunctionType.Exp,
                        )
                        return et

                    def pv(jc, et):
                        nc.tensor.matmul(
                            oacc_a[:, 0:ip],
                            vt[:, jc, 0, :],
                            et[:, 0, 0:ip],
                            start=(jc == 0),
                            stop=(jc == jcp - 1),
                        )
                        nc.tensor.matmul(
                            oacc_b[:, 0:ip],
                            vt[:, jc, 1, :],
                            et[:, 1, 0:ip],
                            start=(jc == 0),
                            stop=(jc == jcp - 1),
                        )

                    # software pipeline: QK^T issued `lookahead` chunks ahead of
                    # PV so PE never waits on the Activation engine's exp
                    ets = []
                    for jc in range(lookahead):
                        ets.append(qk(jc))
                        yield
                    for jc in range(lookahead, jcp):
                        ets.append(qk(jc))
                        pv(jc - lookahead, ets.pop(0))
                        yield
                    for k in range(lookahead):
                        pv(jcp - lookahead + k, ets.pop(0))
                        if k + 1 < lookahead:
                            yield

                    # denominator reciprocals first (DVE), then the PE broadcasts
                    recbs = []
                    for hh, oacc in ((0, oacc_a), (1, oacc_b)):
                        rec = work.tile([1, ip], F32, tag="rec")
                        nc.vector.reciprocal(rec[:, 0:ip], oacc[DH : DH + 1, 0:ip])
                        # fold the query-mask scale into the softmax recip so
                        # O^T comes out pre-scaled by qm
                        recb = work.tile([1, ip], BF16, tag="recb")
                        nc.vector.tensor_tensor(
                            recb[:, 0:ip], rec[:, 0:ip], qmrow_sb[:, 0:ip],
                            mybir.AluOpType.mult,
                        )
                        recbs.append(recb)
                    yield
                    for hh, oacc in ((0, oacc_a), (1, oacc_b)):
                        bps = qpool([128, 2, 512])
                        nc.tensor.matmul(
                            bps[0:DH, 0, 0:ip],
                            ones64[:],
                            recbs[hh][:, 0:ip],
                            start=True,
                            stop=True,
                        )
                        rb = work.tile([DH, ip], F32, tag="rb")
                        nc.vector.tensor_copy(rb[:, 0:ip], bps[0:DH, 0, 0:ip])
                        nc.vector.tensor_tensor(
                            OT_sb[hh * DH : (hh + 1) * DH, hp, 0:ip],
                            oacc[0:DH, 0:ip],
                            rb[:, 0:ip],
                            mybir.AluOpType.mult,
                        )
                        yield

                def c_pair_x2(hp):
                    vt = V_sb[:, :, 2 * hp : 2 * hp + 2, :]
                    oacc_a = accp.tile([DH + 1, 512], F32, tag="oacc")
                    oacc_b = accp.tile([DH + 1, 512], F32, tag="oacc")
                    nb = (jcp + 1) // 2

                    def qk2(b):
                        w = 2 if 2 * b + 1 < jcp else 1
                        sps = ppool.tile([128, 4, 512], F32, tag="sps", bufs=1)
                        for c in range(w):
                            jc = 2 * b + c
                            for hh in range(2):
                                nc.tensor.matmul(
                                    sps[:, 2 * c + hh, 0:ip],
                                    KT_sb[:, hp, jc * 128 : (jc + 1) * 128],
                                    QT2[:, hp, hh, 0:ip],
                                    start=True,
                                    stop=True,
                                )
                        et = etp.tile([128, 4, ip], BF16, tag="et")
                        nc.scalar.activation(
                            et[:, 0 : 2 * w, 0:ip],
                            sps[:, 0 : 2 * w, 0:ip],
                            mybir.ActivationFunctionType.Exp,
                        )
                        return et

                    def pv2(b, et):
                        w = 2 if 2 * b + 1 < jcp else 1
                        for c in range(w):
                            jc = 2 * b + c
                            for hh, oacc in ((0, oacc_a), (1, oacc_b)):
                                nc.tensor.matmul(
                                    oacc[:, 0:ip],
                                    vt[:, jc, hh, :],
                                    et[:, 2 * c + hh, 0:ip],
                                    start=(jc == 0),
                                    stop=(jc == jcp - 1),
                                )

                    ets = [qk2(0)]
                    yield
                    for b in range(1, nb):
                        ets.append(qk2(b))
                        pv2(b - 1, ets.pop(0))
                        yield
                    pv2(nb - 1, ets.pop(0))
                    yield

                    recbs = []
                    for hh, oacc in ((0, oacc_a), (1, oacc_b)):
                        rec = work.tile([1, ip], F32, tag="rec")
                        nc.vector.reciprocal(rec[:, 0:ip], oacc[DH : DH + 1, 0:ip])
                        recb = work.tile([1, ip], BF16, tag="recb")
                        nc.vector.tensor_tensor(
                            recb[:, 0:ip], rec[:, 0:ip], qmrow_sb[:, 0:ip],
                            mybir.AluOpType.mult,
                        )
                        recbs.append(recb)
                    yield
                    for hh, oacc in ((0, oacc_a), (1, oacc_b)):
                        bps = qpool([128, 2, 512])
                        nc.tensor.matmul(
                            bps[0:DH, 0, 0:ip],
                            ones64[:],
                            recbs[hh][:, 0:ip],
                            start=True,
                            stop=True,
                        )
                        rb = work.tile([DH, ip], F32, tag="rb")
                        nc.vector.tensor_copy(rb[:, 0:ip], bps[0:DH, 0, 0:ip])
                        nc.vector.tensor_tensor(
                            OT_sb[hh * DH : (hh + 1) * DH, hp, 0:ip],
                            oacc[0:DH, 0:ip],
                            rb[:, 0:ip],
                            mybir.AluOpType.mult,
                        )
                        yield

                # prefetch the output-projection weights into wq's slot
                wo_sb = wslot.tile([128, EC, D], BF16, tag="w")
                nc.gpsimd.dma_start(
                    wo_sb[:], wout[:].rearrange("(ec p) d -> p ec d", p=128)
                )

                def c_chain(hp0):
                    yield from c_pair(hp0)
                    yield from c_pair(hp0 + 1)

                def x2_chain(hp0):
                    yield from c_pair_x2(hp0)
                    yield from c_pair_x2(hp0 + 1)

                n_c2 = 2 * ((jcp + 1) // 2 + 6)

                n_a = 2 * (DC // A_YIELD_EVERY) * len(jblocks)
                n_c = 2 * (jcp + 4)           # yields per c_chain
                if SCHED == "v3":
                    # a_chunk(0) first so chain 0 can start; then chain 0
                    # overlaps the V projection AND a_chunk(1) (3-way mix).
                    # v_phase is primed 3 chunks ahead and weighted 1:1 with
                    # the chain so V'(jc) is always emitted before chain0's
                    # PV(jc) (emission-order inversion there would deadlock
                    # the PE queue).
                    for _ in a_chunk(0):
                        pass
                    vg = v_phase()
                    for _ in range(3):
                        next(vg)
                    _mix((vg, n_c), (a_chunk(1), n_a), (c_chain(0), n_c))
                    for ep in range(2, EC // 2):
                        _mix((a_chunk(ep), n_a), (c_chain(2 * ep - 2), n_c))
                elif SCHED == "v1":
                    _mix((v_phase(), jcp), (a_chunk(0), n_a))
                    for ep in range(1, EC // 2):
                        _mix((a_chunk(ep), n_a), (c_chain(2 * ep - 2), n_c))
                elif SCHED == "x2":
                    _mix((v_phase(), jcp), (a_chunk(0), n_a))
                    for ep in range(1, EC // 2):
                        _mix((a_chunk(ep), n_a), (x2_chain(2 * ep - 2), n_c2))
                elif SCHED == "dual":
                    # two head-pairs interleaved per chain segment: pair X's
                    # exp hides under pair Y's QK/PV matmuls, breaking the
                    # sps-buffer lockstep that serialized exp with PE work
                    # (measured chain rate ~1.65us/chunk = PE+exp serial).
                    # a_chunk runs outside the chains and shares the sps psum
                    # rotation; the freed banks hold the 4 live oaccs.
                    for g in [v_phase(), a_chunk(0), a_chunk(1), a_chunk(2),
                              a_chunk(3)]:
                        for _ in g:
                            pass
                    for hp0 in range(0, EC, 2):
                        _mix((c_pair(hp0, lookahead=LOOKAHEAD), jcp + 4),
                             (c_pair(hp0 + 1, lookahead=LOOKAHEAD), jcp + 4))
                elif SCHED == "pure":
                    # no interleave: projections run dense, chains run at the
                    # ACT exp rate with an empty PE queue (critical-path test)
                    for g in [v_phase(), a_chunk(0), a_chunk(1), a_chunk(2),
                              a_chunk(3)]:
                        for _ in g:
                            pass
                    for hp0 in range(0, EC - 2, 2):
                        for _ in c_chain(hp0):
                            pass
                else:  # "afirst": all a_chunks before chains; chains unfilled
                    _mix((v_phase(), jcp), (a_chunk(0), n_a))
                    for ep in range(1, EC // 2):
                        for _ in a_chunk(ep):
                            pass
                    for hp0 in range(0, EC - 2, 2):
                        for _ in c_chain(hp0):
                            pass

                # drain the last two attention pairs, interleaved with the
                # output projection for the heads that are already finished
                def d_partial():
                    for ic in range(n_vic):
                        dps = ppool.tile([128, 2, 512], F32, tag=APS_TAG, bufs=APS_BUFS)
                        for db in range(2):
                            for ec in range(EC - 2):
                                nc.tensor.matmul(
                                    dps[:, db, :],
                                    OT_sb[:, ec, ic * 128 : (ic + 1) * 128],
                                    wo_sb[:, ec, db * 512 : (db + 1) * 512],
                                    start=(ec == 0),
                                    stop=False,
                                )
                            nc.tensor.matmul(
                                dps[:, db, :],
                                omqrow_sb[:, ic * 128 : (ic + 1) * 128],
                                ymeanb_sb[:, db * 512 : (db + 1) * 512],
                                start=False,
                                stop=False,
                            )
                            nc.tensor.matmul(
                                dps[:, db, :],
                                qmrow_sb[:, ic * 128 : (ic + 1) * 128],
                                boutr_sb[:, db * 512 : (db + 1) * 512],
                                start=False,
                                stop=True,
                            )
                            yield
                        nc.vector.tensor_copy(Yacc[:, ic, :, :], dps[:])
                        yield

                if SCHED == "x2":
                    _mix((x2_chain(EC - 2), n_c2), (d_partial(), 3 * n_vic))
                elif SCHED != "dual":
                    _mix((c_chain(EC - 2), n_c), (d_partial(), 3 * n_vic))

                # ============ Phase D: finish Y (ec 6,7) + blend ==============
                ic_order = sorted(range(IC), key=lambda ic: ic * 128 < ip)
                for ic in ic_order:
                    has_valid = ic * 128 < ip
                    if has_valid and SCHED == "dual":
                        yps = qpool([128, 2, 512])
                        for db in range(2):
                            for ec in range(EC):
                                nc.tensor.matmul(
                                    yps[:, db, :],
                                    OT_sb[:, ec, ic * 128 : (ic + 1) * 128],
                                    wo_sb[:, ec, db * 512 : (db + 1) * 512],
                                    start=(ec == 0),
                                    stop=False,
                                )
                            nc.tensor.matmul(
                                yps[:, db, :],
                                omqrow_sb[:, ic * 128 : (ic + 1) * 128],
                                ymeanb_sb[:, db * 512 : (db + 1) * 512],
                                start=False,
                                stop=False,
                            )
                            nc.tensor.matmul(
                                yps[:, db, :],
                                qmrow_sb[:, ic * 128 : (ic + 1) * 128],
                                boutr_sb[:, db * 512 : (db + 1) * 512],
                                start=False,
                                stop=True,
                            )
                        for db in range(2):
                            y1 = work.tile([128, 512], BF16, tag="y1", bufs=3)
                            nc.vector.tensor_copy(y1[:], yps[:, db, :])
                            nc.sync.dma_start(
                                y[ic * 128 : (ic + 1) * 128,
                                  db * 512 : (db + 1) * 512],
                                y1[:],
                            )
                    elif has_valid:
                        yps = qpool([128, 2, 512])
                        for db in range(2):
                            for ec in range(EC - 2, EC):
                                nc.tensor.matmul(
                                    yps[:, db, :],
                                    OT_sb[:, ec, ic * 128 : (ic + 1) * 128],
                                    wo_sb[:, ec, db * 512 : (db + 1) * 512],
                                    start=(ec == EC - 2),
                                    stop=(ec == EC - 1),
                                )
                        for db in range(2):
                            y1 = work.tile([128, 512], BF16, tag="y1", bufs=3)
                            nc.vector.tensor_tensor(
                                y1[:], yps[:, db, :], Yacc[:, ic, db, :],
                                mybir.AluOpType.add,
                            )
                            nc.sync.dma_start(
                                y[ic * 128 : (ic + 1) * 128,
                                  db * 512 : (db + 1) * 512],
                                y1[:],
                            )
                    else:
                        bb_a = accp.tile([128, 512], F32, tag="oacc")
                        bb_b = accp.tile([128, 512], F32, tag="oacc")
                        for db, bbps in ((0, bb_a), (1, bb_b)):
                            nc.tensor.matmul(
                                bbps[:],
                                omqrow_sb[:, ic * 128 : (ic + 1) * 128],
                                ymeanb_sb[:, db * 512 : (db + 1) * 512],
                                start=True,
                                stop=False,
                            )
                            nc.tensor.matmul(
                                bbps[:],
                                qmrow_sb[:, ic * 128 : (ic + 1) * 128],
                                boutr_sb[:, db * 512 : (db + 1) * 512],
                                start=False,
                                stop=True,
                            )
                        for db, bb in ((0, bb_a), (1, bb_b)):
                            y1 = work.tile([128, 512], BF16, tag="y1", bufs=3)
                            nc.vector.tensor_copy(y1[:], bb[:])
                            nc.sync.dma_start(
                                y[ic * 128 : (ic + 1) * 128,
                                  db * 512 : (db + 1) * 512],
                                y1[:],
                            )

            for it in range(repeat):
                emit_pass(first=(it == 0))

    nc.compile()
    return nc


def build_nc_stream(jp=J, ip=I):
    """Streaming fallback for jp > JP_RESIDENT_MAX (X^T doesn't fit SBUF):
    the original 5-pass streaming kernel, V through a DRAM round-trip."""
    jcp = jp // 128
    nc = bacc.Bacc()

    txtT = nc.dram_tensor("txtT", [D, I], BF16, kind="ExternalInput")
    imT = nc.dram_tensor("imT", [D, jp], BF16, kind="ExternalInput")
    wq = nc.dram_tensor("wq", [D, E], BF16, kind="ExternalInput")
    wkv = nc.dram_tensor("wkv", [D, 2 * E], BF16, kind="ExternalInput")
    wout = nc.dram_tensor("wout", [E, D], BF16, kind="ExternalInput")
    kvmp = nc.dram_tensor("kvmp", [128, jcp], F32, kind="ExternalInput")
    qmp = nc.dram_tensor("qmp", [128, IC], F32, kind="ExternalInput")
    qmrow = nc.dram_tensor("qmrow", [1, I], BF16, kind="ExternalInput")
    omqrow = nc.dram_tensor("omqrow", [1, I], BF16, kind="ExternalInput")
    ymeanb = nc.dram_tensor("ymeanb", [1, D], BF16, kind="ExternalInput")
    boutr = nc.dram_tensor("boutr", [1, D], BF16, kind="ExternalInput")
    y = nc.dram_tensor("y", [I, D], F32, kind="ExternalOutput")
    vdr = nc.dram_tensor("vdr", [jcp, 128, H, DH], BF16, kind="Internal")

    imTr = imT[:].rearrange("(dc p) j -> p dc j", p=128)

    with tile.TileContext(nc) as tc:
        with (
            tc.tile_pool(name="wpool", bufs=1) as wpool,
            tc.tile_pool(name="wslot", bufs=1) as wslot,
            tc.tile_pool(name="big", bufs=1) as big,
            tc.tile_pool(name="stream", bufs=2) as stream,
            tc.tile_pool(name="work", bufs=3) as work,
            tc.tile_pool(name="etp", bufs=7) as etp,
            tc.tile_pool(name="ppool", bufs=2, space="PSUM") as ppool,
            tc.tile_pool(name="accp", bufs=3, space="PSUM") as accp,
            tc.tile_pool(name="bcp", bufs=1, space="PSUM") as bcp,
        ):
            kvm_sb = wpool.tile([128, jcp], F32)
            nc.sync.dma_start(kvm_sb[:], kvmp[:])
            qmp_sb = wpool.tile([128, IC], F32)
            nc.sync.dma_start(qmp_sb[:], qmp[:])
            qmrow_sb = wpool.tile([1, I], BF16)
            nc.sync.dma_start(qmrow_sb[:], qmrow[:])
            omqrow_sb = wpool.tile([1, I], BF16)
            nc.sync.dma_start(omqrow_sb[:], omqrow[:])
            ymeanb_sb = wpool.tile([1, D], BF16)
            nc.sync.dma_start(ymeanb_sb[:], ymeanb[:])
            boutr_sb = wpool.tile([1, D], BF16)
            nc.sync.dma_start(boutr_sb[:], boutr[:])
            ones64 = wpool.tile([1, DH], BF16)
            nc.vector.memset(ones64[:], 1.0)

            KT_sb = big.tile([128, EC, jp], BF16)
            QT_sb = big.tile([128, EC, I], BF16)
            OT_sb = big.tile([128, EC, I], BF16)
            nc.vector.memset(OT_sb[:], 0.0)

            wq_sb = wslot.tile([128, DC, E], BF16, tag="w")
            nc.gpsimd.dma_start(wq_sb[:], wq[:].rearrange("(dc p) e -> p dc e", p=128))
            ta = stream.tile([128, DC, I], BF16, tag="im")
            nc.gpsimd.dma_start(ta[:], txtT[:].rearrange("(dc p) i -> p dc i", p=128))
            for ep in range(EC // 2):
                ps = ppool.tile([128, 2, 512], F32, tag="s2")
                for eh in range(2):
                    ec = 2 * ep + eh
                    for dc in range(DC):
                        nc.tensor.matmul(
                            ps[:, eh, 0:ip],
                            wq_sb[:, dc, ec * 128 : (ec + 1) * 128],
                            ta[:, dc, 0:ip],
                            start=(dc == 0),
                            stop=(dc == DC - 1),
                        )
                nc.vector.tensor_scalar_mul(
                    QT_sb[:, 2 * ep : 2 * ep + 2, 0:ip], ps[:, :, 0:ip], 0.125
                )

            wv_sb = wslot.tile([128, DC, E], BF16, tag="w")
            nc.gpsimd.dma_start(
                wv_sb[:], wkv[:, E : 2 * E].rearrange("(dc p) e -> p dc e", p=128)
            )
            wk_sb = wslot.tile([128, DC, E], BF16, tag="w")
            nc.gpsimd.dma_start(
                wk_sb[:], wkv[:, 0:E].rearrange("(dc p) e -> p dc e", p=128)
            )

            jblocks = []
            off = 0
            while off < jp:
                w = 512 if jp - off >= 512 else jp - off
                jblocks.append((off, w))
                off += w

            def b_all():
                for off, w in jblocks:
                    imb = stream.tile([128, DC, 512], BF16, tag="ima")
                    nc.gpsimd.dma_start(
                        imb[:, :, 0:w], imTr[:, :, off : off + w]
                    )
                    for jh in range(w // 128):
                        jc = off // 128 + jh
                        ps = ppool.tile([128, 2, 512], F32, tag="s2")
                        for eb in range(2):
                            for dc in range(DC):
                                nc.tensor.matmul(
                                    ps[:, eb, :],
                                    imb[:, dc, jh * 128 : (jh + 1) * 128],
                                    wv_sb[:, dc, eb * 512 : (eb + 1) * 512],
                                    start=(dc == 0),
                                    stop=(dc == DC - 1),
                                )
                        vtmp = work.tile([128, H, DH], BF16, tag="vtmp")
                        nc.vector.tensor_scalar_mul(
                            vtmp[:],
                            ps[:].rearrange("p b (h dh) -> p (b h) dh", dh=DH),
                            kvm_sb[:, jc : jc + 1],
                        )
                        nc.sync.dma_start(vdr[jc, :, :, :], vtmp[:])

            def a_chunk(ep):
                for off, w in jblocks:
                    ima = stream.tile([128, DC, 512], BF16, tag="ima")
                    nc.gpsimd.dma_start(ima[:, :, 0:w], imTr[:, :, off : off + w])
                    ps = ppool.tile([128, 2, 512], F32, tag="s2")
                    for eh in range(2):
                        ec = 2 * ep + eh
                        for dc in range(DC):
                            nc.tensor.matmul(
                                ps[:, eh, 0:w],
                                wk_sb[:, dc, ec * 128 : (ec + 1) * 128],
                                ima[:, dc, 0:w],
                                start=(dc == 0),
                                stop=(dc == DC - 1),
                            )
                    nc.vector.tensor_copy(
                        KT_sb[:, 2 * ep : 2 * ep + 2, off : off + w],
                        ps[:, :, 0:w],
                    )

            def c_pair(hp):
                vtt = stream.tile([128, jcp, 2, DH + 1], BF16, tag="vt")
                nc.vector.tensor_copy(
                    vtt[:, :, 0, DH : DH + 1], kvm_sb[:, :, None]
                )
                nc.vector.tensor_copy(
                    vtt[:, :, 1, DH : DH + 1], kvm_sb[:, :, None]
                )
                for hh in range(2):
                    nc.sync.dma_start(
                        vtt[:, :, hh, 0:DH],
                        vdr[:, :, 2 * hp + hh, :].rearrange("jc p dh -> p jc dh"),
                    )
                vt = vtt
                oacc_a = accp.tile([DH + 1, 512], F32, tag="oacc")
                oacc_b = accp.tile([DH + 1, 512], F32, tag="oacc")
                for jc in range(jcp):
                    sps = ppool.tile([128, 2, 512], F32, tag="s2")
                    nc.tensor.matmul(
                        sps[:, 0, 0:ip],
                        KT_sb[0:DH, hp, jc * 128 : (jc + 1) * 128],
                        QT_sb[0:DH, hp, 0:ip],
                        start=True,
                        stop=True,
                    )
                    nc.tensor.matmul(
                        sps[:, 1, 0:ip],
                        KT_sb[DH:128, hp, jc * 128 : (jc + 1) * 128],
                        QT_sb[DH:128, hp, 0:ip],
                        start=True,
                        stop=True,
                    )
                    et = etp.tile([128, 2, ip], BF16, tag="et")
                    nc.scalar.activation(
                        et[:, :, 0:ip],
                        sps[:, :, 0:ip],
                        mybir.ActivationFunctionType.Exp,
                        scale=kvm_sb[:, jc : jc + 1],
                    )
                    nc.tensor.matmul(
                        oacc_a[:, 0:ip],
                        vt[:, jc, 0, :],
                        et[:, 0, 0:ip],
                        start=(jc == 0),
                        stop=(jc == jcp - 1),
                    )
                    nc.tensor.matmul(
                        oacc_b[:, 0:ip],
                        vt[:, jc, 1, :],
                        et[:, 1, 0:ip],
                        start=(jc == 0),
                        stop=(jc == jcp - 1),
                    )
                for hh, oacc in ((0, oacc_a), (1, oacc_b)):
                    rec = work.tile([1, ip], F32, tag="rec")
                    nc.vector.reciprocal(rec[:, 0:ip], oacc[DH : DH + 1, 0:ip])
                    recb = work.tile([1, ip], BF16, tag="recb")
                    nc.vector.tensor_copy(recb[:, 0:ip], rec[:, 0:ip])
                    bps = bcp.tile([DH, 512], F32, tag="bc")
                    nc.tensor.matmul(
                        bps[:, 0:ip],
                        ones64[:],
                        recb[:, 0:ip],
                        start=True,
                        stop=True,
                    )
                    rb = work.tile([DH, ip], F32, tag="rb")
                    nc.vector.tensor_copy(rb[:, 0:ip], bps[:, 0:ip])
                    nc.vector.tensor_tensor(
                        OT_sb[hh * DH : (hh + 1) * DH, hp, 0:ip],
                        oacc[0:DH, 0:ip],
                        rb[:, 0:ip],
                        mybir.AluOpType.mult,
                    )

            b_all()
            for ep in range(EC // 2):
                a_chunk(ep)
                c_pair(2 * ep)
                c_pair(2 * ep + 1)

            wo_sb = wslot.tile([128, DC, E], BF16, tag="w")
            nc.gpsimd.dma_start(
                wo_sb[:], wout[:].rearrange("(ec p) d -> p ec d", p=128)
            )
            for ic in range(IC):
                has_valid = ic * 128 < ip
                if has_valid:
                    yps = ppool.tile([128, 2, 512], F32, tag="s2")
                    for db in range(2):
                        for ec in range(EC):
                            nc.tensor.matmul(
                                yps[:, db, :],
                                OT_sb[:, ec, ic * 128 : (ic + 1) * 128],
                                wo_sb[:, ec, db * 512 : (db + 1) * 512],
                                start=(ec == 0),
                                stop=(ec == EC - 1),
                            )
                bb_a = accp.tile([128, 512], F32, tag="oacc")
                bb_b = accp.tile([128, 512], F32, tag="oacc")
                for db, bbps in ((0, bb_a), (1, bb_b)):
                    nc.tensor.matmul(
                        bbps[:],
                        omqrow_sb[:, ic * 128 : (ic + 1) * 128],
                        ymeanb_sb[:, db * 512 : (db + 1) * 512],
                        start=True,
                        stop=False,
                    )
                    nc.tensor.matmul(
                        bbps[:],
                        qmrow_sb[:, ic * 128 : (ic + 1) * 128],
                        boutr_sb[:, db * 512 : (db + 1) * 512],
                        start=False,
                        stop=True,
                    )
                y1 = work.tile([128, 2, 512], F32, tag="y1")
                if has_valid:
                    nc.vector.tensor_scalar_mul(
                        y1[:], yps[:], qmp_sb[:, ic : ic + 1]
                    )
                    nc.vector.tensor_tensor(
                        y1[:, 0, :], bb_a[:], y1[:, 0, :], mybir.AluOpType.add
                    )
                    nc.vector.tensor_tensor(
                        y1[:, 1, :], bb_b[:], y1[:, 1, :], mybir.AluOpType.add
                    )
                else:
                    nc.vector.tensor_copy(y1[:, 0, :], bb_a[:])
                    nc.vector.tensor_copy(y1[:, 1, :], bb_b[:])
                nc.sync.dma_start(
                    y[ic * 128 : (ic + 1) * 128, :],
                    y1[:].rearrange("p b d -> p (b d)"),
                )

    nc.compile()
    return nc


_NC_CACHE = {}


def _get_nc(jp, ip=I, repeat=1):
    key = (jp, ip, repeat)
    if key not in _NC_CACHE:
        if jp <= JP_RESIDENT_MAX:
            _NC_CACHE[key] = build_nc(jp, ip, repeat)
        else:
            assert repeat == 1
            _NC_CACHE[key] = build_nc_stream(jp, ip)
    return _NC_CACHE[key]


def _q8(x):
    return np.clip(np.asarray(x, np.float32), -240.0, 240.0).astype(F8)


def prep_inputs(txt, image, kv_mask, q_mask, Wq, Wkv, Wout, bout):
    f32 = np.float32
    Wq = np.asarray(Wq, dtype=f32)
    Wkv = np.asarray(Wkv, dtype=f32)
    Wout = np.asarray(Wout, dtype=f32)
    bout = np.asarray(bout, dtype=f32)
    kvc = kv_mask.sum(axis=1).max()
    qc = q_mask.sum(axis=1).max()
    jp = max(512, int(-(-kvc // 128)) * 128)
    ip = max(256, int(-(-qc // 16)) * 16)
    jcp = jp // 128
    fast = jp <= JP_RESIDENT_MAX
    wq_s = Wq.astype(BF)
    wkv_s = Wkv.astype(BF)
    wout_s = Wout.astype(BF)
    in_maps = []
    perms = []
    for b in range(B):
        kvm = kv_mask[b].astype(bool)
        qm = q_mask[b].astype(bool)
        nkv = int(kvm.sum())
        imTc = np.zeros((D, jp), dtype=BF)
        imTc[:, :nkv] = np.ascontiguousarray(image[b][kvm].T).astype(BF)
        kvmp = np.zeros(jp, dtype=f32)
        kvmp[:nkv] = 1.0
        perm = np.argsort(~qm, kind="stable")
        perms.append(perm)
        qmperm = qm[perm].astype(f32)
        xmean = image[b].astype(f32).mean(axis=0)
        vmean = xmean @ Wkv[:, E:]
        ymb = vmean @ Wout + bout
        blend_s = 1.0
        txtTc = np.ascontiguousarray(txt[b][perm].T)
        in_maps.append(
            {
                "txtT": txtTc.astype(BF),
                "imT": imTc,
                "wq": wq_s,
                "wkv": wkv_s,
                "wout": wout_s,
                "kvmp": np.ascontiguousarray(kvmp.reshape(jcp, 128).T),
                "qmp": np.ascontiguousarray(qmperm.reshape(IC, 128).T),
                "qmrow": qmperm[None, :].astype(BF),
                "omqrow": (1.0 - qmperm)[None, :].astype(BF),
                "ymeanb": (ymb * blend_s)[None, :].astype(BF),
                "boutr": (bout * blend_s)[None, :].astype(BF),
            }
        )
    return in_maps, perms, jp, ip


def run(inputs, trace=False):
    in_maps, perms, jp, ip = prep_inputs(**inputs)
    nc = _get_nc(jp, ip)
    res = run_bass_kernel_spmd(
        nc, in_maps, core_ids=list(range(B)), trace=trace,
        **({"trace_cores": [0]} if trace else {}),
    )
    out = np.empty((B, I, D), dtype=np.float32)
    descale = 1.0
    for b in range(B):
        out[b][perms[b]] = np.asarray(res.results[b]["y"], np.float32) * descale
    return out, res


def kernel(**inputs):
    out, _ = run(inputs, trace=False)
    return out


# revision 4
# speedup vs baseline: 1.0152x; 1.0095x over previous
"""Cross-attention (txt queries -> image kv) Trainium2 Bass kernel, v3.

v1 structure (data-parallel over batch, kv compaction, valid-first q
permutation, ymeanb blend, SBUF-resident X^T, softmax denominator fused as
V-tile column 64) with two fixes found by HW microbenchmarking:

  * QK^T runs at K=128 instead of K=64: Q^T is stored twice with the
    even-head / odd-head partition halves zeroed (QT2[...,0/1,:]), so each
    head's S^T matmul contracts over all 128 partitions (the extra half is
    zeros). K=64 matmuls HW-measure 234ns each (LDWEIGHTS not hidden);
    K=128 measure 117ns — the ~64 zero-row FLOPs are free.

  * psum->SBUF copies for K^T/V run on the Vector engine (DVE) rather than
    the Activation engine, keeping ACT free for the softmax exps (HW A/B:
    317us vs 329us per pass). An alternative schedule that interleaved the
    V projection into the first attention chain measured slower (380us) —
    the per-chunk PE->DVE->PE interlock it creates outweighs the fill; the
    v1 phase ordering is kept (SCHED="v1").

All matmuls bf16 (fp8 was measured end-to-end and rejected: with random
data the attention output is a cancelling average ~40x smaller than v, so
every fp8 path's ~3.5% quantization noise lands full-strength on the
output — each path alone costs 1.7-3.8e-2 of the 2e-2 error budget).
"""

import ml_dtypes
import numpy as np

import concourse.bass as bass
from concourse import bacc
import concourse.mybir as mybir
import concourse.tile as tile
from concourse.bass_utils import run_bass_kernel_spmd

B, I, J, D, E = 8, 512, 4096, 1024, 1024
H, DH = 16, 64
JC = J // 128   # 32
IC = I // 128   # 4
DC = D // 128   # 8
EC = E // 128   # 8
F32 = mybir.dt.float32
BF16 = mybir.dt.bfloat16
FP8 = mybir.dt.float8e4
DR = mybir.MatmulPerfMode.DoubleRow
BF = ml_dtypes.bfloat16
F8 = ml_dtypes.float8_e4m3

WS = 64.0          # host weight pre-scale (wq, wk, wv, wout)
OS = WS * WS       # output-projection result scale -> host divides by 4096

SCHED = "v1"           # "x2": 2-chunk fused exps; "dual"; "v1"; ...
ETP_BUFS = 8 if SCHED == "dual" else (3 if SCHED == "x2" else 6)
A_YIELD_EVERY = 4      # a_chunk emits a yield every N contraction matmuls
COPIES_ON_ACT = False  # psum->SBUF KT/V copies on Act (True) or DVE (False)
LOOKAHEAD = 2          # qk chunks issued ahead of pv in c_pair
ACCP_BUFS = 4 if SCHED == "dual" else 2
SPS4 = SCHED == "x2"   # qk psum is one [128,4,512] tile (4 banks, bufs=1)
PPOOL_BUFS = 2 if SCHED == "dual" else 3
JP_RESIDENT_MAX = 2816  # X^T fully SBUF-resident up to this padded kv length


def _mix(*streams):
    """Proportionally interleave step generators (PE program-order shaping)."""
    live = [[g, n, 0] for g, n in streams if n > 0]
    while live:
        g, n, done = min(live, key=lambda s: s[2] / s[1])
        try:
            next(g)
            for s in live:
                if s[0] is g:
                    s[2] += 1
        except StopIteration:
            live = [s for s in live if s[0] is not g]


APS_TAG = "sps" if SCHED == "dual" else "aps"
APS_BUFS = 2 if SCHED == "dual" else 1
# In x2 mode the remaining non-qk psum users share the aps slot (bufs=1).


def build_nc(jp, ip=I, repeat=1, reload_inputs=True):
    jcp = jp // 128
    nc = bacc.Bacc()

    txtT = nc.dram_tensor("txtT", [D, I], BF16, kind="ExternalInput")
    imT = nc.dram_tensor("imT", [D, jp], BF16, kind="ExternalInput")
    wq = nc.dram_tensor("wq", [D, E], BF16, kind="ExternalInput")
    wkv = nc.dram_tensor("wkv", [D, 2 * E], BF16, kind="ExternalInput")
    wout = nc.dram_tensor("wout", [E, D], BF16, kind="ExternalInput")
    kvmp = nc.dram_tensor("kvmp", [128, jcp], F32, kind="ExternalInput")
    qmp = nc.dram_tensor("qmp", [128, IC], F32, kind="ExternalInput")
    qmrow = nc.dram_tensor("qmrow", [1, I], BF16, kind="ExternalInput")
    omqrow = nc.dram_tensor("omqrow", [1, I], BF16, kind="ExternalInput")
    ymeanb = nc.dram_tensor("ymeanb", [1, D], BF16, kind="ExternalInput")
    boutr = nc.dram_tensor("boutr", [1, D], BF16, kind="ExternalInput")
    y = nc.dram_tensor("y", [I, D], BF16, kind="ExternalOutput")

    jblocks = []
    off = 0
    while off < jp:
        w = 512 if jp - off >= 512 else jp - off
        jblocks.append((off, w))
        off += w

    with tile.TileContext(nc) as tc:
        with (
            tc.tile_pool(name="wpool", bufs=1) as wpool,      # small resident
            tc.tile_pool(name="wslot", bufs=3) as wslot,      # wq/wv/wk live together
            tc.tile_pool(name="big", bufs=1) as big,          # imr, K^T, V, Q^T, O^T
            tc.tile_pool(name="work", bufs=3) as work,        # small working tiles
            tc.tile_pool(name="etp", bufs=ETP_BUFS) as etp,
            tc.tile_pool(name="ppool", bufs=PPOOL_BUFS, space="PSUM") as ppool,
            tc.tile_pool(name="accp", bufs=ACCP_BUFS, space="PSUM") as accp,
        ):
            # ---- one-time allocations (shared across in-NEFF passes) ----
            ta = wpool.tile([128, DC, I], BF16)
            tar = txtT[:].rearrange("(dc p) i -> p dc i", p=128)
            wqr = wq[:].rearrange("(dc p) e -> p dc e", p=128)
            kvm_sb = wpool.tile([128, jcp], F32)
            qmp_sb = wpool.tile([128, IC], F32)
            qmrow_sb = wpool.tile([1, I], BF16)
            omqrow_sb = wpool.tile([1, I], BF16)
            ymeanb_sb = wpool.tile([1, D], BF16)
            ones64 = wpool.tile([1, DH], BF16)
            nc.vector.memset(ones64[:], 1.0)
            imr = big.tile([128, DC, jp], BF16)
            KT_sb = big.tile([128, EC, jp], BF16)
            n_vic = -(-ip // 128)         # i-blocks with any valid query
            ipp = n_vic * 128             # ip padded to whole i-blocks
            # QT2[:, ec, 0, :] has odd-head partitions (64:128) zeroed;
            # QT2[:, ec, 1, :] has even-head partitions (0:64) zeroed —
            # so per-head QK^T contracts over the full 128 partitions.
            QT2 = big.tile([128, EC, 2, ip], BF16)
            nc.vector.memset(QT2[:], 0.0)
            OT_sb = big.tile([128, EC, ipp], BF16)
            nc.vector.memset(OT_sb[:], 0.0)
            Yacc = None if SCHED == "dual" else big.tile([128, n_vic, 2, 512], BF16)
            V_sb = big.tile([128, jcp, H, DH + 1], BF16)
            imTr = imT[:].rearrange("(dc p) j -> p dc j", p=128)

            def qpool(shape):
                if SPS4:
                    qp = ppool.tile(shape, F32, tag="aps", bufs=1, name="qp")
                else:
                    qp = ppool.tile(shape, F32, tag="sps", bufs=2, name="qp")
                return qp

            def emit_pass(first=True):
                # phase Q's operands go first — every DMA transfer
                # serializes through the DMA engines, so these gate PE
                wq_sb = wslot.tile([128, DC, E], BF16, tag="w")
                if first or reload_inputs:
                    nc.sync.dma_start(ta[:], tar[:])
                # wq split along E: phase-Q iteration ep only reads cols
                # [ep*256, (ep+1)*256) — first quarter unblocks PE early
                for q in range(4):
                    nc.scalar.dma_start(
                        wq_sb[:, :, q * 256 : (q + 1) * 256],
                        wqr[:, :, q * 256 : (q + 1) * 256],
                    )
                nc.sync.dma_start(kvm_sb[:], kvmp[:])
                nc.sync.dma_start(qmp_sb[:], qmp[:])
                nc.sync.dma_start(qmrow_sb[:], qmrow[:])
                nc.sync.dma_start(omqrow_sb[:], omqrow[:])
                nc.sync.dma_start(ymeanb_sb[:], ymeanb[:])
                nc.vector.tensor_copy(
                    V_sb[:, :, :, DH : DH + 1],
                    kvm_sb[:, :, None, None].to_broadcast([128, jcp, H, 1]),
                )
                wv_sb = wslot.tile([128, DC, E], BF16, tag="w")
                nc.scalar.dma_start(
                    wv_sb[:],
                    wkv[:, E : 2 * E].rearrange("(dc p) e -> p dc e", p=128),
                )
                if first or reload_inputs:
                    for off, w in [(o, min(512, jp - o)) for o in range(0, jp, 512)]:
                        nc.scalar.dma_start(
                            imr[:, :, off : off + w], imTr[:, :, off : off + w]
                        )
                wk_sb = wslot.tile([128, DC, E], BF16, tag="w")
                nc.sync.dma_start(
                    wk_sb[:], wkv[:, 0:E].rearrange("(dc p) e -> p dc e", p=128)
                )

                # ======= Phase Q: Q^T = Wq^T @ T^T, scaled 1/(8*WS) =========
                for ep in range(EC // 2):
                    ps = qpool([128, 2, 512])
                    for eh in range(2):
                        ec = 2 * ep + eh
                        for dc in range(DC):
                            nc.tensor.matmul(
                                ps[:, eh, 0:ip],
                                wq_sb[:, dc, ec * 128 : (ec + 1) * 128],
                                ta[:, dc, 0:ip],
                                start=(dc == 0),
                                stop=(dc == DC - 1),
                            )
                    # split per head parity into the two zero-padded copies
                    nc.vector.tensor_scalar_mul(
                        QT2[0:DH, 2 * ep : 2 * ep + 2, 0, 0:ip],
                        ps[0:DH, :, 0:ip], 0.125,
                    )
                    nc.vector.tensor_scalar_mul(
                        QT2[DH:128, 2 * ep : 2 * ep + 2, 1, 0:ip],
                        ps[DH:128, :, 0:ip], 0.125,
                    )

                # ===== Phase V: V' = (X Wv) * kvm (x WS), from resident X^T ==
                def v_phase():
                    for jc in range(jcp):
                        ps = qpool([128, 2, 512])
                        for eb in range(2):
                            for dc in range(DC):
                                nc.tensor.matmul(
                                    ps[:, eb, :],
                                    imr[:, dc, jc * 128 : (jc + 1) * 128],
                                    wv_sb[:, dc, eb * 512 : (eb + 1) * 512],
                                    start=(dc == 0),
                                    stop=(dc == DC - 1),
                                )
                        if COPIES_ON_ACT:
                            nc.scalar.activation(
                                V_sb[:, jc, :, 0:DH],
                                ps[:].rearrange("p b (h dh) -> p (b h) dh", dh=DH),
                                mybir.ActivationFunctionType.Copy,
                                scale=kvm_sb[:, jc : jc + 1],
                            )
                        else:
                            nc.vector.tensor_scalar_mul(
                                V_sb[:, jc, :, 0:DH],
                                ps[:].rearrange("p b (h dh) -> p (b h) dh", dh=DH),
                                kvm_sb[:, jc : jc + 1],
                            )
                        yield

                # ====== Main loop: K^T chunks pipelined against attention ====
                def a_chunk(ep):
                    # K^T rows for ec chunks 2ep, 2ep+1, from resident X^T
                    for off, w in jblocks:
                        ps = ppool.tile([128, 2, 512], F32, tag=APS_TAG, bufs=APS_BUFS)
                        for eh in range(2):
                            ec = 2 * ep + eh
                            for dc in range(DC):
                                nc.tensor.matmul(
                                    ps[:, eh, 0:w],
                                    wk_sb[:, dc, ec * 128 : (ec + 1) * 128],
                                    imr[:, dc, off : off + w],
                                    start=(dc == 0),
                                    stop=(dc == DC - 1),
                                )
                                if (dc + 1) % A_YIELD_EVERY == 0 and dc != DC - 1:
                                    yield
                            yield
                        if COPIES_ON_ACT:
                            nc.scalar.activation(
                                KT_sb[:, 2 * ep : 2 * ep + 2, off : off + w],
                                ps[:, :, 0:w],
                                mybir.ActivationFunctionType.Copy,
                            )
                        else:
                            nc.vector.tensor_copy(
                                KT_sb[:, 2 * ep : 2 * ep + 2, off : off + w],
                                ps[:, :, 0:w],
                            )

                def c_pair(hp, accs=None, lookahead=None):
                    if lookahead is None:
                        lookahead = LOOKAHEAD
                    vt = V_sb[:, :, 2 * hp : 2 * hp + 2, :]
                    if accs is None:
                        oacc_a = accp.tile([DH + 1, 512], F32, tag="oacc")
                        oacc_b = accp.tile([DH + 1, 512], F32, tag="oacc")
                    else:
                        oacc_a, oacc_b = accs

                    def qk(jc):
                        sps = ppool.tile([128, 2, 512], F32, tag="sps", bufs=2)
                        for hh in range(2):
                            nc.tensor.matmul(
                                sps[:, hh, 0:ip],
                                KT_sb[:, hp, jc * 128 : (jc + 1) * 128],
                                QT2[:, hp, hh, 0:ip],
                                start=True,
                                stop=True,
                            )
                        et = etp.tile([128, 2, ip], BF16, tag="et")
                        # no kvm scale: compacted K^T is zero at padded j, so
                        # S=0 there and exp(0)=1 is annihilated by V'=0 plus
                        # the kvm denominator column (HW-verified bit-equal)
                        nc.scalar.activation(
                            et[:, :, 0:ip],
                            sps[:, :, 0:ip],
                            mybir.ActivationFunctionType.Exp,
                        )
                        return et

                    def pv(jc, et):
                        nc.tensor.matmul(
                            oacc_a[:, 0:ip],
                            vt[:, jc, 0, :],
                            et[:, 0, 0:ip],
                            start=(jc == 0),
                            stop=(jc == jcp - 1),
                        )
                        nc.tensor.matmul(
                            oacc_b[:, 0:ip],
                            vt[:, jc, 1, :],
                            et[:, 1, 0:ip],
                            start=(jc == 0),
                            stop=(jc == jcp - 1),
                        )

                    # software pipeline: QK^T issued `lookahead` chunks ahead of
                    # PV so PE never waits on the Activation engine's exp
                    ets = []
                    for jc in range(lookahead):
                        ets.append(qk(jc))
                        yield
                    for jc in range(lookahead, jcp):
                        ets.append(qk(jc))
                        pv(jc - lookahead, ets.pop(0))
                        yield
                    for k in range(lookahead):
                        pv(jcp - lookahead + k, ets.pop(0))
                        if k + 1 < lookahead:
                            yield

                    # denominator reciprocals first (DVE), then the PE broadcasts
                    recbs = []
                    for hh, oacc in ((0, oacc_a), (1, oacc_b)):
                        rec = work.tile([1, ip], F32, tag="rec")
                        nc.vector.reciprocal(rec[:, 0:ip], oacc[DH : DH + 1, 0:ip])
                        # fold the query-mask scale into the softmax recip so
                        # O^T comes out pre-scaled by qm
                        recb = work.tile([1, ip], BF16, tag="recb")
                        nc.vector.tensor_tensor(
                            recb[:, 0:ip], rec[:, 0:ip], qmrow_sb[:, 0:ip],
                            mybir.AluOpType.mult,
                        )
                        recbs.append(recb)
                    yield
                    for hh, oacc in ((0, oacc_a), (1, oacc_b)):
                        bps = qpool([128, 2, 512])
                        nc.tensor.matmul(
                            bps[0:DH, 0, 0:ip],
                            ones64[:],
                            recbs[hh][:, 0:ip],
                            start=True,
                            stop=True,
                        )
                        rb = work.tile([DH, ip], F32, tag="rb")
                        nc.vector.tensor_copy(rb[:, 0:ip], bps[0:DH, 0, 0:ip])
                        nc.vector.tensor_tensor(
                            OT_sb[hh * DH : (hh + 1) * DH, hp, 0:ip],
                            oacc[0:DH, 0:ip],
                            rb[:, 0:ip],
                            mybir.AluOpType.mult,
                        )
                        yield

                def c_pair_x2(hp):
                    vt = V_sb[:, :, 2 * hp : 2 * hp + 2, :]
                    oacc_a = accp.tile([DH + 1, 512], F32, tag="oacc")
                    oacc_b = accp.tile([DH + 1, 512], F32, tag="oacc")
                    nb = (jcp + 1) // 2

                    def qk2(b):
                        w = 2 if 2 * b + 1 < jcp else 1
                        sps = ppool.tile([128, 4, 512], F32, tag="sps", bufs=1)
                        for c in range(w):
                            jc = 2 * b + c
                            for hh in range(2):
                                nc.tensor.matmul(
                                    sps[:, 2 * c + hh, 0:ip],
                                    KT_sb[:, hp, jc * 128 : (jc + 1) * 128],
                                    QT2[:, hp, hh, 0:ip],
                                    start=True,
                                    stop=True,
                                )
                        et = etp.tile([128, 4, ip], BF16, tag="et")
                        nc.scalar.activation(
                            et[:, 0 : 2 * w, 0:ip],
                            sps[:, 0 : 2 * w, 0:ip],
                            mybir.ActivationFunctionType.Exp,
                        )
                        return et

                    def pv2(b, et):
                        w = 2 if 2 * b + 1 < jcp else 1
                        for c in range(w):
                            jc = 2 * b + c
                            for hh, oacc in ((0, oacc_a), (1, oacc_b)):
                                nc.tensor.matmul(
                                    oacc[:, 0:ip],
                                    vt[:, jc, hh, :],
                                    et[:, 2 * c + hh, 0:ip],
                                    start=(jc == 0),
                                    stop=(jc == jcp - 1),
                                )

                    ets = [qk2(0)]
                    yield
                    for b in range(1, nb):
                        ets.append(qk2(b))
                        pv2(b - 1, ets.pop(0))
                        yield
                    pv2(nb - 1, ets.pop(0))
                    yield

                    recbs = []
                    for hh, oacc in ((0, oacc_a), (1, oacc_b)):
                        rec = work.tile([1, ip], F32, tag="rec")
                        nc.vector.reciprocal(rec[:, 0:ip], oacc[DH : DH + 1, 0:ip])
                        recb = work.tile([1, ip], BF16, tag="recb")
                        nc.vector.tensor_tensor(
                            recb[:, 0:ip], rec[:, 0:ip], qmrow_sb[:, 0:ip],
                            mybir.AluOpType.mult,
                        )
                        recbs.append(recb)
                    yield
                    for hh, oacc in ((0, oacc_a), (1, oacc_b)):
                        bps = qpool([128, 2, 512])
                        nc.tensor.matmul(
                            bps[0:DH, 0, 0:ip],
                            ones64[:],
                            recbs[hh][:, 0:ip],
                            start=True,
                            stop=True,
                        )
                        rb = work.tile([DH, ip], F32, tag="rb")
                        nc.vector.tensor_copy(rb[:, 0:ip], bps[0:DH, 0, 0:ip])
                        nc.vector.tensor_tensor(
                            OT_sb[hh * DH : (hh + 1) * DH, hp, 0:ip],
                            oacc[0:DH, 0:ip],
                            rb[:, 0:ip],
                            mybir.AluOpType.mult,
                        )
                        yield

                # prefetch the output-projection weights into wq's slot
                wo_sb = wslot.tile([128, EC, D], BF16, tag="w")
                nc.gpsimd.dma_start(
                    wo_sb[:], wout[:].rearrange("(ec p) d -> p ec d", p=128)
                )

                def c_chain(hp0):
                    yield from c_pair(hp0)
                    yield from c_pair(hp0 + 1)

                def x2_chain(hp0):
                    yield from c_pair_x2(hp0)
                    yield from c_pair_x2(hp0 + 1)

                n_c2 = 2 * ((jcp + 1) // 2 + 6)

                n_a = 2 * (DC // A_YIELD_EVERY) * len(jblocks)
                n_c = 2 * (jcp + 4)           # yields per c_chain
                if SCHED == "v3":
                    # a_chunk(0) first so chain 0 can start; then chain 0
                    # overlaps the V projection AND a_chunk(1) (3-way mix).
                    # v_phase is primed 3 chunks ahead and weighted 1:1 with
                    # the chain so V'(jc) is always emitted before chain0's
                    # PV(jc) (emission-order inversion there would deadlock
                    # the PE queue).
                    for _ in a_chunk(0):
                        pass
                    vg = v_phase()
                    for _ in range(3):
                        next(vg)
                    _mix((vg, n_c), (a_chunk(1), n_a), (c_chain(0), n_c))
                    for ep in range(2, EC // 2):
                        _mix((a_chunk(ep), n_a), (c_chain(2 * ep - 2), n_c))
                elif SCHED == "v1":
                    _mix((v_phase(), jcp), (a_chunk(0), n_a))
                    for ep in range(1, EC // 2):
                        _mix((a_chunk(ep), n_a), (c_chain(2 * ep - 2), n_c))
                elif SCHED == "x2":
                    _mix((v_phase(), jcp), (a_chunk(0), n_a))
                    for ep in range(1, EC // 2):
                        _mix((a_chunk(ep), n_a), (x2_chain(2 * ep - 2), n_c2))
                elif SCHED == "dual":
                    # two head-pairs interleaved per chain segment: pair X's
                    # exp hides under pair Y's QK/PV matmuls, breaking the
                    # sps-buffer lockstep that serialized exp with PE work
                    # (measured chain rate ~1.65us/chunk = PE+exp serial).
                    # a_chunk runs outside the chains and shares the sps psum
                    # rotation; the freed banks hold the 4 live oaccs.
                    for g in [v_phase(), a_chunk(0), a_chunk(1), a_chunk(2),
                              a_chunk(3)]:
                        for _ in g:
                            pass
                    for hp0 in range(0, EC, 2):
                        _mix((c_pair(hp0, lookahead=LOOKAHEAD), jcp + 4),
                             (c_pair(hp0 + 1, lookahead=LOOKAHEAD), jcp + 4))
                elif SCHED == "pure":
                    # no interleave: projections run dense, chains run at the
                    # ACT exp rate with an empty PE queue (critical-path test)
                    for g in [v_phase(), a_chunk(0), a_chunk(1), a_chunk(2),
                              a_chunk(3)]:
                        for _ in g:
                            pass
                    for hp0 in range(0, EC - 2, 2):
                        for _ in c_chain(hp0):
                            pass
                else:  # "afirst": all a_chunks before chains; chains unfilled
                    _mix((v_phase(), jcp), (a_chunk(0), n_a))
                    for ep in range(1, EC // 2):
                        for _ in a_chunk(ep):
                            pass
                    for hp0 in range(0, EC - 2, 2):
                        for _ in c_chain(hp0):
                            pass

                # drain the last two attention pairs, interleaved with the
                # output projection for the heads that are already finished
                def d_partial():
                    for ic in range(n_vic):
                        dps = ppool.tile([128, 2, 512], F32, tag=APS_TAG, bufs=APS_BUFS)
                        for db in range(2):
                            for ec in range(EC - 2):
                                nc.tensor.matmul(
                                    dps[:, db, :],
                                    OT_sb[:, ec, ic * 128 : (ic + 1) * 128],
                                    wo_sb[:, ec, db * 512 : (db + 1) * 512],
                                    start=(ec == 0),
                                    stop=False,
                                )
                            nc.tensor.matmul(
                                dps[:, db, :],
                                omqrow_sb[:, ic * 128 : (ic + 1) * 128],
                                ymeanb_sb[:, db * 512 : (db + 1) * 512],
                                start=False,
                                stop=True,
                            )
                            yield
                        nc.vector.tensor_copy(Yacc[:, ic, :, :], dps[:])
                        yield

                if SCHED == "x2":
                    _mix((x2_chain(EC - 2), n_c2), (d_partial(), 3 * n_vic))
                elif SCHED != "dual":
                    _mix((c_chain(EC - 2), n_c), (d_partial(), 3 * n_vic))

                # ============ Phase D: finish Y (ec 6,7) + blend ==============
                ic_order = sorted(range(IC), key=lambda ic: ic * 128 < ip)
                for ic in ic_order:
                    has_valid = ic * 128 < ip
                    if has_valid and SCHED == "dual":
                        yps = qpool([128, 2, 512])
                        for db in range(2):
                            for ec in range(EC):
                                nc.tensor.matmul(
                                    yps[:, db, :],
                                    OT_sb[:, ec, ic * 128 : (ic + 1) * 128],
                                    wo_sb[:, ec, db * 512 : (db + 1) * 512],
                                    start=(ec == 0),
                                    stop=False,
                                )
                            nc.tensor.matmul(
                                yps[:, db, :],
                                omqrow_sb[:, ic * 128 : (ic + 1) * 128],
                                ymeanb_sb[:, db * 512 : (db + 1) * 512],
                                start=False,
                                stop=False,
                            )
                            nc.tensor.matmul(
                                yps[:, db, :],
                                qmrow_sb[:, ic * 128 : (ic + 1) * 128],
                                boutr_sb[:, db * 512 : (db + 1) * 512],
                                start=False,
                                stop=True,
                            )
                        for db in range(2):
                            y1 = work.tile([128, 512], BF16, tag="y1", bufs=3)
                            nc.vector.tensor_copy(y1[:], yps[:, db, :])
                            nc.sync.dma_start(
                                y[ic * 128 : (ic + 1) * 128,
                                  db * 512 : (db + 1) * 512],
                                y1[:],
                            )
                    elif has_valid:
                        yps = qpool([128, 2, 512])
                        for db in range(2):
                            for ec in range(EC - 2, EC):
                                nc.tensor.matmul(
                                    yps[:, db, :],
                                    OT_sb[:, ec, ic * 128 : (ic + 1) * 128],
                                    wo_sb[:, ec, db * 512 : (db + 1) * 512],
                                    start=(ec == EC - 2),
                                    stop=(ec == EC - 1),
                                )
                        for db in range(2):
                            y1 = work.tile([128, 512], BF16, tag="y1", bufs=3)
                            nc.vector.tensor_tensor(
                                y1[:], yps[:, db, :], Yacc[:, ic, db, :],
                                mybir.AluOpType.add,
                            )
                            nc.sync.dma_start(
                                y[ic * 128 : (ic + 1) * 128,
                                  db * 512 : (db + 1) * 512],
                                y1[:],
                            )
                    else:
                        bb_a = accp.tile([128, 512], F32, tag="oacc")
                        bb_b = accp.tile([128, 512], F32, tag="oacc")
                        for db, bbps in ((0, bb_a), (1, bb_b)):
                            nc.tensor.matmul(
                                bbps[:],
                                omqrow_sb[:, ic * 128 : (ic + 1) * 128],
                                ymeanb_sb[:, db * 512 : (db + 1) * 512],
                                start=True,
                                stop=True,
                            )
                        for db, bb in ((0, bb_a), (1, bb_b)):
                            y1 = work.tile([128, 512], BF16, tag="y1", bufs=3)
                            nc.vector.tensor_copy(y1[:], bb[:])
                            nc.sync.dma_start(
                                y[ic * 128 : (ic + 1) * 128,
                                  db * 512 : (db + 1) * 512],
                                y1[:],
                            )

            for it in range(repeat):
                emit_pass(first=(it == 0))

    nc.compile()
    return nc


def build_nc_stream(jp=J, ip=I):
    """Streaming fallback for jp > JP_RESIDENT_MAX (X^T doesn't fit SBUF):
    the original 5-pass streaming kernel, V through a DRAM round-trip."""
    jcp = jp // 128
    nc = bacc.Bacc()

    txtT = nc.dram_tensor("txtT", [D, I], BF16, kind="ExternalInput")
    imT = nc.dram_tensor("imT", [D, jp], BF16, kind="ExternalInput")
    wq = nc.dram_tensor("wq", [D, E], BF16, kind="ExternalInput")
    wkv = nc.dram_tensor("wkv", [D, 2 * E], BF16, kind="ExternalInput")
    wout = nc.dram_tensor("wout", [E, D], BF16, kind="ExternalInput")
    kvmp = nc.dram_tensor("kvmp", [128, jcp], F32, kind="ExternalInput")
    qmp = nc.dram_tensor("qmp", [128, IC], F32, kind="ExternalInput")
    qmrow = nc.dram_tensor("qmrow", [1, I], BF16, kind="ExternalInput")
    omqrow = nc.dram_tensor("omqrow", [1, I], BF16, kind="ExternalInput")
    ymeanb = nc.dram_tensor("ymeanb", [1, D], BF16, kind="ExternalInput")
    boutr = nc.dram_tensor("boutr", [1, D], BF16, kind="ExternalInput")
    y = nc.dram_tensor("y", [I, D], F32, kind="ExternalOutput")
    vdr = nc.dram_tensor("vdr", [jcp, 128, H, DH], BF16, kind="Internal")

    imTr = imT[:].rearrange("(dc p) j -> p dc j", p=128)

    with tile.TileContext(nc) as tc:
        with (
            tc.tile_pool(name="wpool", bufs=1) as wpool,
            tc.tile_pool(name="wslot", bufs=1) as wslot,
            tc.tile_pool(name="big", bufs=1) as big,
            tc.tile_pool(name="stream", bufs=2) as stream,
            tc.tile_pool(name="work", bufs=3) as work,
            tc.tile_pool(name="etp", bufs=7) as etp,
            tc.tile_pool(name="ppool", bufs=2, space="PSUM") as ppool,
            tc.tile_pool(name="accp", bufs=3, space="PSUM") as accp,
            tc.tile_pool(name="bcp", bufs=1, space="PSUM") as bcp,
        ):
            kvm_sb = wpool.tile([128, jcp], F32)
            nc.sync.dma_start(kvm_sb[:], kvmp[:])
            qmp_sb = wpool.tile([128, IC], F32)
            nc.sync.dma_start(qmp_sb[:], qmp[:])
            qmrow_sb = wpool.tile([1, I], BF16)
            nc.sync.dma_start(qmrow_sb[:], qmrow[:])
            omqrow_sb = wpool.tile([1, I], BF16)
            nc.sync.dma_start(omqrow_sb[:], omqrow[:])
            ymeanb_sb = wpool.tile([1, D], BF16)
            nc.sync.dma_start(ymeanb_sb[:], ymeanb[:])
            boutr_sb = wpool.tile([1, D], BF16)
            nc.sync.dma_start(boutr_sb[:], boutr[:])
            ones64 = wpool.tile([1, DH], BF16)
            nc.vector.memset(ones64[:], 1.0)

            KT_sb = big.tile([128, EC, jp], BF16)
            QT_sb = big.tile([128, EC, I], BF16)
            OT_sb = big.tile([128, EC, I], BF16)
            nc.vector.memset(OT_sb[:], 0.0)

            wq_sb = wslot.tile([128, DC, E], BF16, tag="w")
            nc.gpsimd.dma_start(wq_sb[:], wq[:].rearrange("(dc p) e -> p dc e", p=128))
            ta = stream.tile([128, DC, I], BF16, tag="im")
            nc.gpsimd.dma_start(ta[:], txtT[:].rearrange("(dc p) i -> p dc i", p=128))
            for ep in range(EC // 2):
                ps = ppool.tile([128, 2, 512], F32, tag="s2")
                for eh in range(2):
                    ec = 2 * ep + eh
                    for dc in range(DC):
                        nc.tensor.matmul(
                            ps[:, eh, 0:ip],
                            wq_sb[:, dc, ec * 128 : (ec + 1) * 128],
                            ta[:, dc, 0:ip],
                            start=(dc == 0),
                            stop=(dc == DC - 1),
                        )
                nc.vector.tensor_scalar_mul(
                    QT_sb[:, 2 * ep : 2 * ep + 2, 0:ip], ps[:, :, 0:ip], 0.125
                )

            wv_sb = wslot.tile([128, DC, E], BF16, tag="w")
            nc.gpsimd.dma_start(
                wv_sb[:], wkv[:, E : 2 * E].rearrange("(dc p) e -> p dc e", p=128)
            )
            wk_sb = wslot.tile([128, DC, E], BF16, tag="w")
            nc.gpsimd.dma_start(
                wk_sb[:], wkv[:, 0:E].rearrange("(dc p) e -> p dc e", p=128)
            )

            jblocks = []
            off = 0
            while off < jp:
                w = 512 if jp - off >= 512 else jp - off
                jblocks.append((off, w))
                off += w

            def b_all():
                for off, w in jblocks:
                    imb = stream.tile([128, DC, 512], BF16, tag="ima")
                    nc.gpsimd.dma_start(
                        imb[:, :, 0:w], imTr[:, :, off : off + w]
                    )
                    for jh in range(w // 128):
                        jc = off // 128 + jh
                        ps = ppool.tile([128, 2, 512], F32, tag="s2")
                        for eb in range(2):
                            for dc in range(DC):
                                nc.tensor.matmul(
                                    ps[:, eb, :],
                                    imb[:, dc, jh * 128 : (jh + 1) * 128],
                                    wv_sb[:, dc, eb * 512 : (eb + 1) * 512],
                                    start=(dc == 0),
                                    stop=(dc == DC - 1),
                                )
                        vtmp = work.tile([128, H, DH], BF16, tag="vtmp")
                        nc.vector.tensor_scalar_mul(
                            vtmp[:],
                            ps[:].rearrange("p b (h dh) -> p (b h) dh", dh=DH),
                            kvm_sb[:, jc : jc + 1],
                        )
                        nc.sync.dma_start(vdr[jc, :, :, :], vtmp[:])

            def a_chunk(ep):
                for off, w in jblocks:
                    ima = stream.tile([128, DC, 512], BF16, tag="ima")
                    nc.gpsimd.dma_start(ima[:, :, 0:w], imTr[:, :, off : off + w])
                    ps = ppool.tile([128, 2, 512], F32, tag="s2")
                    for eh in range(2):
                        ec = 2 * ep + eh
                        for dc in range(DC):
                            nc.tensor.matmul(
                                ps[:, eh, 0:w],
                                wk_sb[:, dc, ec * 128 : (ec + 1) * 128],
                                ima[:, dc, 0:w],
                                start=(dc == 0),
                                stop=(dc == DC - 1),
                            )
                    nc.vector.tensor_copy(
                        KT_sb[:, 2 * ep : 2 * ep + 2, off : off + w],
                        ps[:, :, 0:w],
                    )

            def c_pair(hp):
                vtt = stream.tile([128, jcp, 2, DH + 1], BF16, tag="vt")
                nc.vector.tensor_copy(
                    vtt[:, :, 0, DH : DH + 1], kvm_sb[:, :, None]
                )
                nc.vector.tensor_copy(
                    vtt[:, :, 1, DH : DH + 1], kvm_sb[:, :, None]
                )
                for hh in range(2):
                    nc.sync.dma_start(
                        vtt[:, :, hh, 0:DH],
                        vdr[:, :, 2 * hp + hh, :].rearrange("jc p dh -> p jc dh"),
                    )
                vt = vtt
                oacc_a = accp.tile([DH + 1, 512], F32, tag="oacc")
                oacc_b = accp.tile([DH + 1, 512], F32, tag="oacc")
                for jc in range(jcp):
                    sps = ppool.tile([128, 2, 512], F32, tag="s2")
                    nc.tensor.matmul(
                        sps[:, 0, 0:ip],
                        KT_sb[0:DH, hp, jc * 128 : (jc + 1) * 128],
                        QT_sb[0:DH, hp, 0:ip],
                        start=True,
                        stop=True,
                    )
                    nc.tensor.matmul(
                        sps[:, 1, 0:ip],
                        KT_sb[DH:128, hp, jc * 128 : (jc + 1) * 128],
                        QT_sb[DH:128, hp, 0:ip],
                        start=True,
                        stop=True,
                    )
                    et = etp.tile([128, 2, ip], BF16, tag="et")
                    nc.scalar.activation(
                        et[:, :, 0:ip],
                        sps[:, :, 0:ip],
                        mybir.ActivationFunctionType.Exp,
                        scale=kvm_sb[:, jc : jc + 1],
                    )
                    nc.tensor.matmul(
                        oacc_a[:, 0:ip],
                        vt[:, jc, 0, :],
                        et[:, 0, 0:ip],
                        start=(jc == 0),
                        stop=(jc == jcp - 1),
                    )
                    nc.tensor.matmul(
                        oacc_b[:, 0:ip],
                        vt[:, jc, 1, :],
                        et[:, 1, 0:ip],
                        start=(jc == 0),
                        stop=(jc == jcp - 1),
                    )
                for hh, oacc in ((0, oacc_a), (1, oacc_b)):
                    rec = work.tile([1, ip], F32, tag="rec")
                    nc.vector.reciprocal(rec[:, 0:ip], oacc[DH : DH + 1, 0:ip])
                    recb = work.tile([1, ip], BF16, tag="recb")
                    nc.vector.tensor_copy(recb[:, 0:ip], rec[:, 0:ip])
                    bps = bcp.tile([DH, 512], F32, tag="bc")
                    nc.tensor.matmul(
                        bps[:, 0:ip],
                        ones64[:],
                        recb[:, 0:ip],
                        start=True,
                        stop=True,
                    )
                    rb = work.tile([DH, ip], F32, tag="rb")
                    nc.vector.tensor_copy(rb[:, 0:ip], bps[:, 0:ip])
                    nc.vector.tensor_tensor(
                        OT_sb[hh * DH : (hh + 1) * DH, hp, 0:ip],
                        oacc[0:DH, 0:ip],
                        rb[:, 0:ip],
                        mybir.AluOpType.mult,
                    )

            b_all()
            for ep in range(EC // 2):
                a_chunk(ep)
                c_pair(2 * ep)
                c_pair(2 * ep + 1)

            wo_sb = wslot.tile([128, DC, E], BF16, tag="w")
            nc.gpsimd.dma_start(
                wo_sb[:], wout[:].rearrange("(ec p) d -> p ec d", p=128)
            )
            for ic in range(IC):
                has_valid = ic * 128 < ip
                if has_valid:
                    yps = ppool.tile([128, 2, 512], F32, tag="s2")
                    for db in range(2):
                        for ec in range(EC):
                            nc.tensor.matmul(
                                yps[:, db, :],
                                OT_sb[:, ec, ic * 128 : (ic + 1) * 128],
                                wo_sb[:, ec, db * 512 : (db + 1) * 512],
                                start=(ec == 0),
                                stop=(ec == EC - 1),
                            )
                bb_a = accp.tile([128, 512], F32, tag="oacc")
                bb_b = accp.tile([128, 512], F32, tag="oacc")
                for db, bbps in ((0, bb_a), (1, bb_b)):
                    nc.tensor.matmul(
                        bbps[:],
                        omqrow_sb[:, ic * 128 : (ic + 1) * 128],
                        ymeanb_sb[:, db * 512 : (db + 1) * 512],
                        start=True,
                        stop=False,
                    )
                    nc.tensor.matmul(
                        bbps[:],
                        qmrow_sb[:, ic * 128 : (ic + 1) * 128],
                        boutr_sb[:, db * 512 : (db + 1) * 512],
                        start=False,
                        stop=True,
                    )
                y1 = work.tile([128, 2, 512], F32, tag="y1")
                if has_valid:
                    nc.vector.tensor_scalar_mul(
                        y1[:], yps[:], qmp_sb[:, ic : ic + 1]
                    )
                    nc.vector.tensor_tensor(
                        y1[:, 0, :], bb_a[:], y1[:, 0, :], mybir.AluOpType.add
                    )
                    nc.vector.tensor_tensor(
                        y1[:, 1, :], bb_b[:], y1[:, 1, :], mybir.AluOpType.add
                    )
                else:
                    nc.vector.tensor_copy(y1[:, 0, :], bb_a[:])
                    nc.vector.tensor_copy(y1[:, 1, :], bb_b[:])
                nc.sync.dma_start(
                    y[ic * 128 : (ic + 1) * 128, :],
                    y1[:].rearrange("p b d -> p (b d)"),
                )

    nc.compile()
    return nc


_NC_CACHE = {}


def _get_nc(jp, ip=I, repeat=1):
    key = (jp, ip, repeat)
    if key not in _NC_CACHE:
        if jp <= JP_RESIDENT_MAX:
            _NC_CACHE[key] = build_nc(jp, ip, repeat)
        else:
            assert repeat == 1
            _NC_CACHE[key] = build_nc_stream(jp, ip)
    return _NC_CACHE[key]


def _q8(x):
    return np.clip(np.asarray(x, np.float32), -240.0, 240.0).astype(F8)


def prep_inputs(txt, image, kv_mask, q_mask, Wq, Wkv, Wout, bout):
    f32 = np.float32
    Wq = np.asarray(Wq, dtype=f32)
    Wkv = np.asarray(Wkv, dtype=f32)
    Wout = np.asarray(Wout, dtype=f32)
    bout = np.asarray(bout, dtype=f32)
    kvc = kv_mask.sum(axis=1).max()
    qc = q_mask.sum(axis=1).max()
    jp = max(512, int(-(-kvc // 128)) * 128)
    ip = max(256, int(-(-qc // 16)) * 16)
    jcp = jp // 128
    fast = jp <= JP_RESIDENT_MAX
    wq_s = Wq.astype(BF)
    wkv_s = Wkv.astype(BF)
    wout_s = Wout.astype(BF)
    in_maps = []
    perms = []
    for b in range(B):
        kvm = kv_mask[b].astype(bool)
        qm = q_mask[b].astype(bool)
        nkv = int(kvm.sum())
        imTc = np.zeros((D, jp), dtype=BF)
        imTc[:, :nkv] = np.ascontiguousarray(image[b][kvm].T).astype(BF)
        kvmp = np.zeros(jp, dtype=f32)
        kvmp[:nkv] = 1.0
        perm = np.argsort(~qm, kind="stable")
        perms.append(perm)
        qmperm = qm[perm].astype(f32)
        xmean = image[b].astype(f32).mean(axis=0)
        vmean = xmean @ Wkv[:, E:]
        ymb = vmean @ Wout + (0.0 if fast else bout)
        blend_s = 1.0
        txtTc = np.ascontiguousarray(txt[b][perm].T)
        in_maps.append(
            {
                "txtT": txtTc.astype(BF),
                "imT": imTc,
                "wq": wq_s,
                "wkv": wkv_s,
                "wout": wout_s,
                "kvmp": np.ascontiguousarray(kvmp.reshape(jcp, 128).T),
                "qmp": np.ascontiguousarray(qmperm.reshape(IC, 128).T),
                "qmrow": qmperm[None, :].astype(BF),
                "omqrow": (1.0 - qmperm)[None, :].astype(BF),
                "ymeanb": (ymb * blend_s)[None, :].astype(BF),
                "boutr": (bout * blend_s)[None, :].astype(BF),
            }
        )
    return in_maps, perms, jp, ip


def run(inputs, trace=False):
    in_maps, perms, jp, ip = prep_inputs(**inputs)
    nc = _get_nc(jp, ip)
    res = run_bass_kernel_spmd(
        nc, in_maps, core_ids=list(range(B)), trace=trace,
        **({"trace_cores": [0]} if trace else {}),
    )
    out = np.empty((B, I, D), dtype=np.float32)
    for b in range(B):
        out[b][perms[b]] = np.asarray(res.results[b]["y"], np.float32)
    if jp <= JP_RESIDENT_MAX:
        out += np.asarray(inputs["bout"], np.float32)[None, None, :]
    return out, res


def kernel(**inputs):
    out, _ = run(inputs, trace=False)
    return out
